# revision 2
# baseline (speedup 1.0000x reference)
"""2-layer GAT (PyG-style GATConv x2 + log_softmax) on 8 Trainium2 NeuronCores.

Sharding: dst-node sharding (each core owns N/8 destination nodes and all
edges into them). Node features (x) are replicated; each core computes the
full layer-1 node transform, so the only cross-core exchange is one
AllGather of the small layer-2 per-node table between layers.

Edge phase per core: edges sorted by dst block (128 dst nodes per block),
tiles of 128 edges. Per tile: one indirect DMA gathers the [as1|h] rows of
the edge sources from a DRAM table; ad1[dst] is reconstructed on-chip with
a one-hot matmul (no second gather); attention weights p = exp(lrelu(as+ad))
are computed chunked per block; a one-hot aggregation matmul accumulates
[p | p*h] into the per-block PSUM, which is then normalized (segment
softmax) without materializing per-edge alphas.
"""
import sys
sys.path.insert(0, '/opt/trn_rl_repo')
if '/root/.axon_site' not in sys.path:
    sys.path.insert(0, '/root/.axon_site')

import math
import numpy as np

import concourse.bass as bass
import concourse.bacc as bacc
import concourse.tile as tile
from concourse import mybir
from concourse import bass_utils

F16 = mybir.dt.float16
F32 = mybir.dt.float32
I32 = mybir.dt.int32
AX = mybir.AxisListType
ALU = mybir.AluOpType
ACTF = mybir.ActivationFunctionType

CORES = 8
P = 128


def _ap(t, off, dims):
    """AP over pool tile t: partition dim from the tile + given free dims."""
    base = t[:]
    return bass.AP(base.tensor, base.offset + off, [list(base.ap[0])] + [list(d) for d in dims])


def _build_program(N, F, H, C, NC, T_B, NTAB, NBLK, NODE_BLKS, ESHIFT):
    """Build the SPMD Bass program (identical across cores)."""
    HC = H * C
    OWNPAD = NBLK * P
    NT = int(sum(T_B))
    G1W = 8 + HC            # [as1 | h] row width (72)
    G2W = 1 + NC + 1 + 6    # [as2 | h2 | ad2 | pad] = 48
    G2R = 1 + NC            # gathered part of a g2 row (41)
    W2K = HC + 1            # 65

    nc = bacc.Bacc("TRN2", target_bir_lowering=False, debug=False,
                   num_devices=CORES)

    xT = nc.dram_tensor("xT", [F, NODE_BLKS * P], F16, kind="ExternalInput").ap()
    w1aug = nc.dram_tensor("w1aug", [F, 8 + HC + 8], F16, kind="ExternalInput").ap()
    b1aug = nc.dram_tensor("b1aug", [1, 8 + HC + 8], F16, kind="ExternalInput").ap()
    w2aug = nc.dram_tensor("w2aug", [W2K, G2W], F16, kind="ExternalInput").ap()
    ones1 = nc.dram_tensor("ones1", [1, P], F16, kind="ExternalInput").ap()
    iotaf = nc.dram_tensor("iotaf", [P, P], F32, kind="ExternalInput").ap()
    ident = nc.dram_tensor("ident", [P, P], F16, kind="ExternalInput").ap()
    sidx1 = nc.dram_tensor("sidx1", [P, NT], I32, kind="ExternalInput").ap()
    sidx2 = nc.dram_tensor("sidx2", [P, NT], I32, kind="ExternalInput").ap()
    dstc = nc.dram_tensor("dstc", [P, NT], F32, kind="ExternalInput").ap()
    out = nc.dram_tensor("out", [OWNPAD, NC], F32, kind="ExternalOutput").ap()

    with tile.TileContext(nc) as tc:
        with tc.tile_pool(name="const", bufs=1) as cp, \
             tc.tile_pool(name="xp", bufs=4) as xp, \
             tc.tile_pool(name="stp", bufs=3) as stp, \
             tc.tile_pool(name="gp", bufs=2) as gp, \
             tc.tile_pool(name="ohp", bufs=2) as ohp, \
             tc.tile_pool(name="vp", bufs=2) as vp, \
             tc.tile_pool(name="ohtp", bufs=3) as ohtp, \
             tc.tile_pool(name="ep", bufs=2) as ep, \
             tc.tile_pool(name="psA", bufs=2, space="PSUM") as psA, \
             tc.tile_pool(name="psB", bufs=2, space="PSUM") as psB, \
             tc.tile_pool(name="psT", bufs=2, space="PSUM") as psT, \
             tc.tile_pool(name="dram", bufs=1, space="DRAM") as dp:

            g1tab = dp.tile([NTAB, G1W], F16)
            g2own = dp.tile([OWNPAD, G2W], F16)
            g2full = dp.tile([CORES * OWNPAD, G2W], F16, addr_space="Shared")

            # ---- resident constants -------------------------------------
            iota_sb = cp.tile([P, P], F32)
            nc.sync.dma_start(out=iota_sb[:], in_=iotaf[:, :])
            ident_sb = cp.tile([P, P], F16)
            nc.sync.dma_start(out=ident_sb[:], in_=ident[:, :])
            w1a0 = cp.tile([P, 8 + HC + 8], F16)
            nc.sync.dma_start(out=w1a0[:], in_=w1aug[0:P, :])
            w1a1 = cp.tile([P, 8 + HC + 8], F16)
            nc.sync.dma_start(out=w1a1[:], in_=w1aug[P:2 * P, :])
            b1a = cp.tile([1, 8 + HC + 8], F16)
            nc.sync.dma_start(out=b1a[:], in_=b1aug[:, :])
            w2a = cp.tile([W2K, G2W], F16)
            nc.sync.dma_start(out=w2a[:], in_=w2aug[:, :])
            ones_sb = cp.tile([1, P], F16)
            nc.sync.dma_start(out=ones_sb[:], in_=ones1[:, :])
            si1 = cp.tile([P, NT], I32)
            nc.sync.dma_start(out=si1[:], in_=sidx1[:, :])
            si2 = cp.tile([P, NT], I32)
            nc.sync.dma_start(out=si2[:], in_=sidx2[:, :])
            dst_sb = cp.tile([P, NT], F32)
            nc.sync.dma_start(out=dst_sb[:], in_=dstc[:, :])
            ad1own = cp.tile([P, NBLK * 8], F16)
            ad2own = cp.tile([P, NBLK], F16)
            lhsT65 = cp.tile([W2K, P], F16)
            nc.vector.memset(lhsT65[:], 0.0)
            nc.vector.memset(lhsT65[HC:W2K, :], 1.0)
            zcol = cp.tile([P, 1], F32)
            nc.vector.memset(zcol[:], 0.0)
            scol = cp.tile([P, 1], F32)
            nc.vector.memset(scol[:], ESHIFT)

            # ---- node phase: g1 table for every node --------------------
            for b in range(NODE_BLKS):
                x0 = xp.tile([P, P], F16, tag="x0")
                nc.sync.dma_start(out=x0[:], in_=xT[0:P, b * P:(b + 1) * P])
                x1 = xp.tile([P, P], F16, tag="x1")
                nc.sync.dma_start(out=x1[:], in_=xT[P:2 * P, b * P:(b + 1) * P])
                ps = psA.tile([P, 8 + HC + 8], F32, tag="psA")
                nc.tensor.matmul(out=ps[:], lhsT=x0[:], rhs=w1a0[:], start=True, stop=False)
                nc.tensor.matmul(out=ps[:], lhsT=x1[:], rhs=w1a1[:], start=False, stop=False)
                nc.tensor.matmul(out=ps[:], lhsT=ones_sb[:], rhs=b1a[:], start=False, stop=True)
                st = stp.tile([P, G1W], F16, tag="g1st")
                nc.vector.tensor_copy(out=st[:], in_=ps[:, 0:G1W])
                nc.sync.dma_start(out=g1tab[b * P:(b + 1) * P, :], in_=st[:])
                if b < NBLK:
                    nc.vector.tensor_copy(out=ad1own[:, b * 8:(b + 1) * 8],
                                          in_=ps[:, G1W:G1W + 8])

            # ---- layer 1 edge phase + epilogue --------------------------
            t0 = 0
            for b in range(NBLK):
                tb = int(T_B[b])
                g1c = gp.tile([P, tb * G1W], F16, tag="g1c")
                for k in range(tb):
                    nc.gpsimd.indirect_dma_start(
                        out=g1c[:, k * G1W:(k + 1) * G1W], out_offset=None,
                        in_=g1tab[:, :],
                        in_offset=bass.IndirectOffsetOnAxis(
                            ap=si1[:, t0 + k:t0 + k + 1], axis=0))
                ohc = ohp.tile([P, tb * P], F16, tag="ohc")
                nc.vector.tensor_tensor(
                    out=_ap(ohc, 0, [[P, tb], [1, P]]),
                    in0=_ap(iota_sb, 0, [[0, tb], [1, P]]),
                    in1=_ap(dst_sb, t0, [[1, tb], [0, P]]),
                    op=ALU.is_equal)
                adps = psB.tile([P, tb * 8], F32, tag="psB")
                for k in range(tb):
                    ohT_ps = psT.tile([P, P], F16, tag="psT")
                    nc.tensor.transpose(out=ohT_ps[:], in_=ohc[:, k * P:(k + 1) * P],
                                        identity=ident_sb[:])
                    ohT = ohtp.tile([P, P], F16, tag="ohT")
                    nc.vector.tensor_copy(out=ohT[:], in_=ohT_ps[:])
                    nc.tensor.matmul(out=adps[:, k * 8:(k + 1) * 8], lhsT=ohT[:],
                                     rhs=ad1own[:, b * 8:(b + 1) * 8],
                                     start=True, stop=True)
                ech = ep.tile([P, tb * 8], F32, tag="ech")
                nc.vector.tensor_tensor(
                    out=_ap(ech, 0, [[8, tb], [1, 8]]),
                    in0=_ap(g1c, 0, [[G1W, tb], [1, 8]]),
                    in1=_ap(adps, 0, [[8, tb], [1, 8]]),
                    op=ALU.add)
                lrch = ep.tile([P, tb * 8], F32, tag="lrch")
                nc.vector.scalar_tensor_tensor(out=lrch[:], in0=ech[:], scalar=0.2,
                                               in1=ech[:], op0=ALU.mult, op1=ALU.max)
                pch = ep.tile([P, tb * 8], F32, tag="pch")
                nc.scalar.activation(pch[:], lrch[:], ACTF.Exp, bias=scol[:, 0:1])
                vc = vp.tile([P, tb * G1W], F16, tag="vc")
                nc.vector.tensor_copy(
                    out=_ap(vc, 0, [[G1W, tb], [1, 8]]),
                    in_=_ap(pch, 0, [[8, tb], [1, 8]]))
                nc.vector.tensor_tensor(
                    out=_ap(vc, 8, [[G1W, tb], [8, H], [1, C]]),
                    in0=_ap(g1c, 8, [[G1W, tb], [8, H], [1, C]]),
                    in1=_ap(pch, 0, [[8, tb], [1, H], [0, C]]),
                    op=ALU.mult)
                psagg = psA.tile([P, G1W], F32, tag="psA")
                for k in range(tb):
                    nc.tensor.matmul(out=psagg[:], lhsT=ohc[:, k * P:(k + 1) * P],
                                     rhs=vc[:, k * G1W:(k + 1) * G1W],
                                     start=(k == 0), stop=(k == tb - 1))
                # epilogue: segment-softmax normalize + ELU + layer-2 node xform
                ssb = ep.tile([P, 8], F32, tag="ssb")
                nc.vector.tensor_scalar_add(out=ssb[:], in0=psagg[:, 0:8], scalar1=1e-16)
                sinv = ep.tile([P, 8], F32, tag="sinv")
                nc.vector.reciprocal(out=sinv[:], in_=ssb[:])
                h1f = ep.tile([P, HC], F32, tag="h1f")
                nc.vector.tensor_tensor(
                    out=_ap(h1f, 0, [[C, H], [1, C]]),
                    in0=_ap(psagg, 8, [[C, H], [1, C]]),
                    in1=_ap(sinv, 0, [[1, H], [0, C]]),
                    op=ALU.mult)
                t1 = ep.tile([P, HC], F32, tag="t1")
                nc.vector.tensor_scalar_min(out=t1[:], in0=h1f[:], scalar1=0.0)
                t2 = ep.tile([P, HC], F32, tag="t2")
                nc.scalar.activation(t2[:], t1[:], ACTF.Exp, bias=zcol[:, 0:1])
                t3 = ep.tile([P, HC], F32, tag="t3")
                nc.vector.tensor_scalar_max(out=t3[:], in0=h1f[:], scalar1=0.0)
                t4 = ep.tile([P, HC], F32, tag="t4")
                nc.vector.tensor_tensor(out=t4[:], in0=t2[:], in1=t3[:], op=ALU.add)
                h1e = ep.tile([P, HC], F16, tag="h1e")
                nc.vector.tensor_scalar_add(out=h1e[:], in0=t4[:], scalar1=-1.0)
                trp = psT.tile([HC, P], F16, tag="psT")
                nc.tensor.transpose(out=trp[:], in_=h1e[:], identity=ident_sb[:])
                nc.vector.tensor_copy(out=lhsT65[0:HC, :], in_=trp[:])
                ps2 = psB.tile([P, G2W], F32, tag="psB")
                nc.tensor.matmul(out=ps2[:], lhsT=lhsT65[:], rhs=w2a[:],
                                 start=True, stop=True)
                g2st = stp.tile([P, G2W], F16, tag="g2st")
                nc.vector.tensor_copy(out=g2st[:], in_=ps2[:])
                nc.sync.dma_start(out=g2own[b * P:(b + 1) * P, :], in_=g2st[:])
                nc.vector.tensor_copy(out=ad2own[:, b:b + 1], in_=ps2[:, G2R:G2R + 1])
                t0 += tb

            # ---- halo exchange of layer-2 node table --------------------
            nc.gpsimd.collective_compute(
                "AllGather", ALU.bypass,
                ins=[g2own[:].opt()], outs=[g2full[:].opt()],
                replica_groups=[list(range(CORES))])

            # ---- layer 2 edge phase + epilogue --------------------------
            t0 = 0
            for b in range(NBLK):
                tb = int(T_B[b])
                g2c = gp.tile([P, tb * G2R], F16, tag="g2c")
                for k in range(tb):
                    nc.gpsimd.indirect_dma_start(
                        out=g2c[:, k * G2R:(k + 1) * G2R], out_offset=None,
                        in_=g2full[:, :],
                        in_offset=bass.IndirectOffsetOnAxis(
                            ap=si2[:, t0 + k:t0 + k + 1], axis=0))
                ohc = ohp.tile([P, tb * P], F16, tag="ohc")
                nc.vector.tensor_tensor(
                    out=_ap(ohc, 0, [[P, tb], [1, P]]),
                    in0=_ap(iota_sb, 0, [[0, tb], [1, P]]),
                    in1=_ap(dst_sb, t0, [[1, tb], [0, P]]),
                    op=ALU.is_equal)
                adps = psB.tile([P, tb], F32, tag="psB")
                for k in range(tb):
                    ohT_ps = psT.tile([P, P], F16, tag="psT")
                    nc.tensor.transpose(out=ohT_ps[:], in_=ohc[:, k * P:(k + 1) * P],
                                        identity=ident_sb[:])
                    ohT = ohtp.tile([P, P], F16, tag="ohT")
                    nc.vector.tensor_copy(out=ohT[:], in_=ohT_ps[:])
                    nc.tensor.matmul(out=adps[:, k:k + 1], lhsT=ohT[:],
                                     rhs=ad2own[:, b:b + 1], start=True, stop=True)
                ech = ep.tile([P, tb], F32, tag="ech")
                nc.vector.tensor_tensor(
                    out=ech[:],
                    in0=_ap(g2c, 0, [[G2R, tb]]),
                    in1=adps[:],
                    op=ALU.add)
                lrch = ep.tile([P, tb], F32, tag="lrch")
                nc.vector.scalar_tensor_tensor(out=lrch[:], in0=ech[:], scalar=0.2,
                                               in1=ech[:], op0=ALU.mult, op1=ALU.max)
                pch = ep.tile([P, tb], F32, tag="pch")
                nc.scalar.activation(pch[:], lrch[:], ACTF.Exp, bias=zcol[:, 0:1])
                vc = vp.tile([P, tb * G2R], F16, tag="vc")
                nc.vector.tensor_copy(out=_ap(vc, 0, [[G2R, tb]]), in_=pch[:])
                for k in range(tb):
                    nc.vector.tensor_scalar_mul(
                        out=vc[:, k * G2R + 1:(k + 1) * G2R],
                        in0=g2c[:, k * G2R + 1:(k + 1) * G2R],
                        scalar1=pch[:, k:k + 1])
                psagg = psA.tile([P, G2R], F32, tag="psA")
                for k in range(tb):
                    nc.tensor.matmul(out=psagg[:], lhsT=ohc[:, k * P:(k + 1) * P],
                                     rhs=vc[:, k * G2R:(k + 1) * G2R],
                                     start=(k == 0), stop=(k == tb - 1))
                ssb = ep.tile([P, 1], F32, tag="ssb")
                nc.vector.tensor_scalar_add(out=ssb[:], in0=psagg[:, 0:1], scalar1=1e-16)
                sinv = ep.tile([P, 1], F32, tag="sinv")
                nc.vector.reciprocal(out=sinv[:], in_=ssb[:])
                lg = ep.tile([P, NC], F32, tag="t1")
                nc.vector.tensor_scalar_mul(out=lg[:], in0=psagg[:, 1:1 + NC],
                                            scalar1=sinv[:, 0:1])
                mx = ep.tile([P, 1], F32, tag="mx")
                nc.vector.reduce_max(mx[:], lg[:], axis=AX.X)
                sh = ep.tile([P, NC], F32, tag="t2")
                nc.vector.tensor_scalar_sub(out=sh[:], in0=lg[:], scalar1=mx[:, 0:1])
                ex = ep.tile([P, NC], F32, tag="t3")
                nc.scalar.activation(ex[:], sh[:], ACTF.Exp, bias=zcol[:, 0:1])
                sm = ep.tile([P, 1], F32, tag="sm")
                nc.vector.reduce_sum(sm[:], ex[:], axis=AX.X)
                ls = ep.tile([P, 1], F32, tag="ls")
                nc.scalar.activation(ls[:], sm[:], ACTF.Ln, bias=zcol[:, 0:1])
                ob = ep.tile([P, NC], F32, tag="t4")
                nc.vector.tensor_scalar_sub(out=ob[:], in0=sh[:], scalar1=ls[:, 0:1])
                nc.sync.dma_start(out=out[b * P:(b + 1) * P, :], in_=ob[:])
                t0 += tb

    nc.compile()
    return nc


def _prep(x, edge_src, edge_dst, W1, a1_src, a1_dst, b1, W2, a2_src, a2_dst, b2):
    """Host-side integer preprocessing (graph partitioning) + param folding."""
    N, F = x.shape
    H, C = a1_src.shape
    NC = W2.shape[1]
    HC = H * C
    NOWN = N // CORES
    NBLK = math.ceil(NOWN / P)
    OWNPAD = NBLK * P
    NFOR = N - NOWN
    FBLK = math.ceil(NFOR / P)
    NODE_BLKS = NBLK + FBLK
    NTAB = NODE_BLKS * P

    src_all = np.concatenate([edge_src, np.arange(N, dtype=edge_src.dtype)])
    dst_all = np.concatenate([edge_dst, np.arange(N, dtype=edge_dst.dtype)])

    # per (core, block) edge lists
    core_of = dst_all // NOWN
    per_core = []
    cnt = np.zeros((CORES, NBLK), np.int64)
    for c in range(CORES):
        m = core_of == c
        s, d = src_all[m], dst_all[m] - c * NOWN
        blk = d // P
        order = np.argsort(blk, kind='stable')
        s, d, blk = s[order], d[order], blk[order]
        cnt[c] = np.bincount(blk, minlength=NBLK)
        per_core.append((s, d, blk))
    T_B = np.maximum(1, np.ceil(cnt.max(axis=0) / P).astype(np.int64))
    NT = int(T_B.sum())
    toff = np.concatenate([[0], np.cumsum(T_B)])

    # param folding
    W1r = W1.reshape(F, H, C)
    wsrc = (W1r * a1_src[None]).sum(-1)          # [F, H]
    wdst = (W1r * a1_dst[None]).sum(-1)          # [F, H]
    w1aug = np.concatenate([wsrc, W1, wdst], axis=1).astype(np.float16)   # [F, 8+HC+8]
    b1aug = np.zeros((1, 8 + HC + 8), np.float16)
    b1aug[0, 8:8 + HC] = b1.astype(np.float16)
    G2W = 1 + NC + 1 + 6
    W2K = HC + 1
    w2aug = np.zeros((W2K, G2W), np.float16)
    w2aug[0:HC, 0] = (W2 @ a2_src[0]).astype(np.float16)
    w2aug[0:HC, 1:1 + NC] = W2.astype(np.float16)
    w2aug[0:HC, 1 + NC] = (W2 @ a2_dst[0]).astype(np.float16)
    w2aug[HC, 1:1 + NC] = b2.astype(np.float16)
    ones1 = np.ones((1, P), np.float16)
    iotaf = np.tile(np.arange(P, dtype=np.float32)[None, :], (P, 1))
    ident = np.eye(P, dtype=np.float16)

    xT = np.ascontiguousarray(x.T)               # [F, N] float32

    in_maps = []
    for c in range(CORES):
        own_lo, own_hi = c * NOWN, (c + 1) * NOWN
        # perm: table position -> node
        xTp = np.zeros((F, NTAB), np.float16)
        xTp[:, 0:NOWN] = xT[:, own_lo:own_hi].astype(np.float16)
        fore = np.concatenate([np.arange(0, own_lo), np.arange(own_hi, N)])
        xTp[:, OWNPAD:OWNPAD + NFOR] = xT[:, fore].astype(np.float16)
        # node -> table position
        pos = np.empty(N, np.int64)
        pos[own_lo:own_hi] = np.arange(NOWN)
        pos[fore] = OWNPAD + np.arange(NFOR)

        s, d, blk = per_core[c]
        sidx1 = np.zeros((P, NT), np.int32)
        sidx2 = np.zeros((P, NT), np.int32)
        dstc = np.full((P, NT), -1.0, np.float32)
        bstart = np.concatenate([[0], np.cumsum(np.bincount(blk, minlength=NBLK))])
        for b in range(NBLK):
            eb = slice(bstart[b], bstart[b + 1])
            sb_, db_ = s[eb], d[eb]
            n = len(sb_)
            for k in range(int(T_B[b])):
                lo, hi = k * P, min((k + 1) * P, n)
                if lo >= n:
                    break
                t = toff[b] + k
                m = hi - lo
                sidx1[0:m, t] = pos[sb_[lo:hi]]
                sidx2[0:m, t] = (sb_[lo:hi] // NOWN) * OWNPAD + (sb_[lo:hi] % NOWN)
                dstc[0:m, t] = (db_[lo:hi] % P).astype(np.float32)
        in_maps.append({
            "xT": xTp, "w1aug": w1aug, "b1aug": b1aug, "w2aug": w2aug,
            "ones1": ones1, "iotaf": iotaf.astype(np.float32), "ident": ident,
            "sidx1": sidx1, "sidx2": sidx2, "dstc": dstc,
        })
    meta = dict(N=N, F=F, H=H, C=C, NC=NC, T_B=T_B, NTAB=NTAB, NBLK=NBLK,
                NODE_BLKS=NODE_BLKS, NOWN=NOWN)
    return in_maps, meta


_CACHED = {}


def run(inputs, eshift=-4.0, trace=False):
    in_maps, meta = _prep(**inputs)
    key = (meta["N"], meta["F"], meta["NC"], tuple(meta["T_B"]))
    if key not in _CACHED:
        _CACHED[key] = _build_program(meta["N"], meta["F"], meta["H"], meta["C"],
                                      meta["NC"], meta["T_B"], meta["NTAB"],
                                      meta["NBLK"], meta["NODE_BLKS"], eshift)
    nc = _CACHED[key]
    res = bass_utils.run_bass_kernel_spmd(nc, in_maps,
                                          core_ids=list(range(CORES)),
                                          trace=trace)
    outs = [res.results[c]["out"][:meta["NOWN"]] for c in range(CORES)]
    full = np.concatenate(outs, axis=0).astype(np.float32)
    return full, res


def kernel(**inputs):
    full, _ = run(inputs)
    return full


# revision 3
# speedup vs baseline: 1.0463x; 1.0463x over previous
"""2-layer GAT (PyG-style GATConv x2 + log_softmax) on 8 Trainium2 NeuronCores.

Sharding: dst-node sharding (each core owns N/8 destination nodes and all
edges into them). Node features (x) are replicated; each core computes the
full layer-1 node transform, so the only cross-core exchange is one
AllGather of the small layer-2 per-node table between layers.

Edge phase per core: edges sorted by dst block (128 dst nodes per block),
tiles of 128 edges. Per tile: one indirect DMA gathers the [as1|h] rows of
the edge sources from a DRAM table; ad1[dst] is reconstructed on-chip with
a one-hot matmul (no second gather); attention weights p = exp(lrelu(as+ad))
are computed chunked per block; a one-hot aggregation matmul accumulates
[p | p*h] into the per-block PSUM, which is then normalized (segment
softmax) without materializing per-edge alphas.
"""
import sys
sys.path.insert(0, '/opt/trn_rl_repo')
if '/root/.axon_site' not in sys.path:
    sys.path.insert(0, '/root/.axon_site')

import math
import numpy as np

import concourse.bass as bass
import concourse.bacc as bacc
import concourse.tile as tile
from concourse import mybir
from concourse import bass_utils

F16 = mybir.dt.float16
F32 = mybir.dt.float32
I32 = mybir.dt.int32
AX = mybir.AxisListType
ALU = mybir.AluOpType
ACTF = mybir.ActivationFunctionType

CORES = 8
P = 128


def _ap(t, off, dims):
    """AP over pool tile t: partition dim from the tile + given free dims."""
    base = t[:]
    return bass.AP(base.tensor, base.offset + off, [list(base.ap[0])] + [list(d) for d in dims])


def _build_program(N, F, H, C, NC, T_B, NTAB, NBLK, NODE_BLKS, ESHIFT):
    """Build the SPMD Bass program (identical across cores)."""
    HC = H * C
    OWNPAD = NBLK * P
    NT = int(sum(T_B))
    G1W = 8 + HC            # [as1 | h] row width (72)
    G2W = 1 + NC + 1 + 6    # [as2 | h2 | ad2 | pad] = 48
    G2R = 1 + NC            # gathered part of a g2 row (41)
    W2K = HC + 1            # 65

    nc = bacc.Bacc("TRN2", target_bir_lowering=False, debug=False,
                   num_devices=CORES)

    xT = nc.dram_tensor("xT", [F, NODE_BLKS * P], F16, kind="ExternalInput").ap()
    w1aug = nc.dram_tensor("w1aug", [F, 8 + HC + 8], F16, kind="ExternalInput").ap()
    b1aug = nc.dram_tensor("b1aug", [1, 8 + HC + 8], F16, kind="ExternalInput").ap()
    w2aug = nc.dram_tensor("w2aug", [W2K, G2W], F16, kind="ExternalInput").ap()
    ones1 = nc.dram_tensor("ones1", [1, P], F16, kind="ExternalInput").ap()
    iotaf = nc.dram_tensor("iotaf", [P, P], F32, kind="ExternalInput").ap()
    ident = nc.dram_tensor("ident", [P, P], F16, kind="ExternalInput").ap()
    sidx1 = nc.dram_tensor("sidx1", [P, NT], I32, kind="ExternalInput").ap()
    sidx2 = nc.dram_tensor("sidx2", [P, NT], I32, kind="ExternalInput").ap()
    dstc = nc.dram_tensor("dstc", [P, NT], F32, kind="ExternalInput").ap()
    out = nc.dram_tensor("out", [OWNPAD, NC], F32, kind="ExternalOutput").ap()

    with tile.TileContext(nc) as tc:
        with tc.tile_pool(name="const", bufs=1) as cp, \
             tc.tile_pool(name="xp", bufs=4) as xp, \
             tc.tile_pool(name="stp", bufs=3) as stp, \
             tc.tile_pool(name="gp", bufs=2) as gp, \
             tc.tile_pool(name="ohp", bufs=2) as ohp, \
             tc.tile_pool(name="vp", bufs=2) as vp, \
             tc.tile_pool(name="ohtp", bufs=3) as ohtp, \
             tc.tile_pool(name="ep", bufs=2) as ep, \
             tc.tile_pool(name="psA", bufs=2, space="PSUM") as psA, \
             tc.tile_pool(name="psB", bufs=2, space="PSUM") as psB, \
             tc.tile_pool(name="psT", bufs=2, space="PSUM") as psT, \
             tc.tile_pool(name="dram", bufs=1, space="DRAM") as dp:

            g1tab = dp.tile([NTAB, G1W], F16)
            g2own = dp.tile([OWNPAD, G2W], F16)
            g2full = dp.tile([CORES * OWNPAD, G2W], F16, addr_space="Shared")

            # ---- resident constants -------------------------------------
            iota_sb = cp.tile([P, P], F32)
            nc.sync.dma_start(out=iota_sb[:], in_=iotaf[:, :])
            ident_sb = cp.tile([P, P], F16)
            nc.sync.dma_start(out=ident_sb[:], in_=ident[:, :])
            w1a0 = cp.tile([P, 8 + HC + 8], F16)
            nc.sync.dma_start(out=w1a0[:], in_=w1aug[0:P, :])
            w1a1 = cp.tile([P, 8 + HC + 8], F16)
            nc.sync.dma_start(out=w1a1[:], in_=w1aug[P:2 * P, :])
            b1a = cp.tile([1, 8 + HC + 8], F16)
            nc.sync.dma_start(out=b1a[:], in_=b1aug[:, :])
            w2a = cp.tile([W2K, G2W], F16)
            nc.sync.dma_start(out=w2a[:], in_=w2aug[:, :])
            ones_sb = cp.tile([1, P], F16)
            nc.sync.dma_start(out=ones_sb[:], in_=ones1[:, :])
            si1 = cp.tile([P, NT], I32)
            nc.sync.dma_start(out=si1[:], in_=sidx1[:, :])
            si2 = cp.tile([P, NT], I32)
            nc.sync.dma_start(out=si2[:], in_=sidx2[:, :])
            dst_sb = cp.tile([P, NT], F32)
            nc.sync.dma_start(out=dst_sb[:], in_=dstc[:, :])
            ad1own = cp.tile([P, NBLK * 8], F16)
            ad2own = cp.tile([P, NBLK], F16)
            as1own = cp.tile([P, NBLK * 8], F16)
            h1own = cp.tile([P, NBLK * HC], F16)
            as2own = cp.tile([P, NBLK], F16)
            h2own = cp.tile([P, NBLK * NC], F16)
            lhsT65 = cp.tile([W2K, P], F16)
            nc.vector.memset(lhsT65[:], 0.0)
            nc.vector.memset(lhsT65[HC:W2K, :], 1.0)
            zcol = cp.tile([P, 1], F32)
            nc.vector.memset(zcol[:], 0.0)
            scol = cp.tile([P, 1], F32)
            nc.vector.memset(scol[:], ESHIFT)

            # ---- node phase: g1 table for every node --------------------
            for b in range(NODE_BLKS):
                x0 = xp.tile([P, P], F16, tag="x0")
                nc.sync.dma_start(out=x0[:], in_=xT[0:P, b * P:(b + 1) * P])
                x1 = xp.tile([P, P], F16, tag="x1")
                nc.sync.dma_start(out=x1[:], in_=xT[P:2 * P, b * P:(b + 1) * P])
                ps = psA.tile([P, 8 + HC + 8], F32, tag="psA")
                nc.tensor.matmul(out=ps[:], lhsT=x0[:], rhs=w1a0[:], start=True, stop=False)
                nc.tensor.matmul(out=ps[:], lhsT=x1[:], rhs=w1a1[:], start=False, stop=False)
                nc.tensor.matmul(out=ps[:], lhsT=ones_sb[:], rhs=b1a[:], start=False, stop=True)
                st = stp.tile([P, G1W], F16, tag="g1st")
                nc.vector.tensor_copy(out=st[:], in_=ps[:, 0:G1W])
                nc.sync.dma_start(out=g1tab[b * P:(b + 1) * P, :], in_=st[:])
                if b < NBLK:
                    nc.vector.tensor_copy(out=ad1own[:, b * 8:(b + 1) * 8],
                                          in_=ps[:, G1W:G1W + 8])
                    nc.vector.tensor_copy(out=as1own[:, b * 8:(b + 1) * 8],
                                          in_=ps[:, 0:8])
                    nc.vector.tensor_copy(out=h1own[:, b * HC:(b + 1) * HC],
                                          in_=ps[:, 8:8 + HC])

            # ---- layer 1 edge phase + epilogue --------------------------
            t0 = 0
            for b in range(NBLK):
                tb = int(T_B[b])
                g1c = gp.tile([P, tb * G1W], F16, tag="g1c")
                for k in range(tb):
                    nc.gpsimd.indirect_dma_start(
                        out=g1c[:, k * G1W:(k + 1) * G1W], out_offset=None,
                        in_=g1tab[:, :],
                        in_offset=bass.IndirectOffsetOnAxis(
                            ap=si1[:, t0 + k:t0 + k + 1], axis=0))
                ohc = ohp.tile([P, tb * P], F16, tag="ohc")
                nc.vector.tensor_tensor(
                    out=_ap(ohc, 0, [[P, tb], [1, P]]),
                    in0=_ap(iota_sb, 0, [[0, tb], [1, P]]),
                    in1=_ap(dst_sb, t0, [[1, tb], [0, P]]),
                    op=ALU.is_equal)
                adps = psB.tile([P, tb * 8], F32, tag="psB")
                for k in range(tb):
                    ohT_ps = psT.tile([P, P], F16, tag="psT")
                    nc.tensor.transpose(out=ohT_ps[:], in_=ohc[:, k * P:(k + 1) * P],
                                        identity=ident_sb[:])
                    ohT = ohtp.tile([P, P], F16, tag="ohT")
                    nc.vector.tensor_copy(out=ohT[:], in_=ohT_ps[:])
                    nc.tensor.matmul(out=adps[:, k * 8:(k + 1) * 8], lhsT=ohT[:],
                                     rhs=ad1own[:, b * 8:(b + 1) * 8],
                                     start=True, stop=True)
                ech = ep.tile([P, tb * 8], F32, tag="ech")
                nc.vector.tensor_tensor(
                    out=_ap(ech, 0, [[8, tb], [1, 8]]),
                    in0=_ap(g1c, 0, [[G1W, tb], [1, 8]]),
                    in1=_ap(adps, 0, [[8, tb], [1, 8]]),
                    op=ALU.add)
                lrch = ep.tile([P, tb * 8], F32, tag="lrch")
                nc.vector.scalar_tensor_tensor(out=lrch[:], in0=ech[:], scalar=0.2,
                                               in1=ech[:], op0=ALU.mult, op1=ALU.max)
                pch = ep.tile([P, tb * 8], F32, tag="pch")
                nc.scalar.activation(pch[:], lrch[:], ACTF.Exp, bias=scol[:, 0:1])
                vc = vp.tile([P, tb * G1W], F16, tag="vc")
                nc.vector.tensor_copy(
                    out=_ap(vc, 0, [[G1W, tb], [1, 8]]),
                    in_=_ap(pch, 0, [[8, tb], [1, 8]]))
                nc.vector.tensor_tensor(
                    out=_ap(vc, 8, [[G1W, tb], [8, H], [1, C]]),
                    in0=_ap(g1c, 8, [[G1W, tb], [8, H], [1, C]]),
                    in1=_ap(pch, 0, [[8, tb], [1, H], [0, C]]),
                    op=ALU.mult)
                psagg = psA.tile([P, G1W], F32, tag="psA")
                for k in range(tb):
                    nc.tensor.matmul(out=psagg[:], lhsT=ohc[:, k * P:(k + 1) * P],
                                     rhs=vc[:, k * G1W:(k + 1) * G1W],
                                     start=(k == 0), stop=(k == tb - 1))
                # epilogue: self-loop term, segment-softmax normalize, ELU
                es = ep.tile([P, 8], F32, tag="es")
                nc.vector.tensor_tensor(out=es[:], in0=as1own[:, b * 8:(b + 1) * 8],
                                        in1=ad1own[:, b * 8:(b + 1) * 8], op=ALU.add)
                lrs = ep.tile([P, 8], F32, tag="lrs")
                nc.vector.scalar_tensor_tensor(out=lrs[:], in0=es[:], scalar=0.2,
                                               in1=es[:], op0=ALU.mult, op1=ALU.max)
                psf = ep.tile([P, 8], F32, tag="psf")
                nc.scalar.activation(psf[:], lrs[:], ACTF.Exp, bias=scol[:, 0:1])
                st_ = ep.tile([P, 8], F32, tag="st_")
                nc.vector.tensor_tensor(out=st_[:], in0=psagg[:, 0:8], in1=psf[:], op=ALU.add)
                ssb = ep.tile([P, 8], F32, tag="ssb")
                nc.vector.tensor_scalar_add(out=ssb[:], in0=st_[:], scalar1=1e-16)
                sinv = ep.tile([P, 8], F32, tag="sinv")
                nc.vector.reciprocal(out=sinv[:], in_=ssb[:])
                hw = ep.tile([P, HC], F32, tag="hw")
                nc.vector.tensor_tensor(
                    out=_ap(hw, 0, [[C, H], [1, C]]),
                    in0=_ap(h1own, b * HC, [[C, H], [1, C]]),
                    in1=_ap(psf, 0, [[1, H], [0, C]]),
                    op=ALU.mult)
                wf = ep.tile([P, HC], F32, tag="wf")
                nc.vector.tensor_tensor(out=wf[:], in0=psagg[:, 8:8 + HC], in1=hw[:], op=ALU.add)
                h1f = ep.tile([P, HC], F32, tag="h1f")
                nc.vector.tensor_tensor(
                    out=_ap(h1f, 0, [[C, H], [1, C]]),
                    in0=_ap(wf, 0, [[C, H], [1, C]]),
                    in1=_ap(sinv, 0, [[1, H], [0, C]]),
                    op=ALU.mult)
                t1 = ep.tile([P, HC], F32, tag="t1")
                nc.vector.tensor_scalar_min(out=t1[:], in0=h1f[:], scalar1=0.0)
                t2 = ep.tile([P, HC], F32, tag="t2")
                nc.scalar.activation(t2[:], t1[:], ACTF.Exp, bias=zcol[:, 0:1])
                t3 = ep.tile([P, HC], F32, tag="t3")
                nc.vector.tensor_scalar_max(out=t3[:], in0=h1f[:], scalar1=0.0)
                t4 = ep.tile([P, HC], F32, tag="t4")
                nc.vector.tensor_tensor(out=t4[:], in0=t2[:], in1=t3[:], op=ALU.add)
                h1e = ep.tile([P, HC], F16, tag="h1e")
                nc.vector.tensor_scalar_add(out=h1e[:], in0=t4[:], scalar1=-1.0)
                trp = psT.tile([HC, P], F16, tag="psT")
                nc.tensor.transpose(out=trp[:], in_=h1e[:], identity=ident_sb[:])
                nc.vector.tensor_copy(out=lhsT65[0:HC, :], in_=trp[:])
                ps2 = psB.tile([P, G2W], F32, tag="psB")
                nc.tensor.matmul(out=ps2[:], lhsT=lhsT65[:], rhs=w2a[:],
                                 start=True, stop=True)
                g2st = stp.tile([P, G2W], F16, tag="g2st")
                nc.vector.tensor_copy(out=g2st[:], in_=ps2[:])
                nc.sync.dma_start(out=g2own[b * P:(b + 1) * P, :], in_=g2st[:])
                nc.vector.tensor_copy(out=ad2own[:, b:b + 1], in_=ps2[:, G2R:G2R + 1])
                nc.vector.tensor_copy(out=as2own[:, b:b + 1], in_=ps2[:, 0:1])
                nc.vector.tensor_copy(out=h2own[:, b * NC:(b + 1) * NC], in_=ps2[:, 1:1 + NC])
                t0 += tb

            # ---- halo exchange of layer-2 node table --------------------
            nc.gpsimd.collective_compute(
                "AllGather", ALU.bypass,
                ins=[g2own[:].opt()], outs=[g2full[:].opt()],
                replica_groups=[list(range(CORES))])

            # ---- layer 2 edge phase + epilogue --------------------------
            t0 = 0
            for b in range(NBLK):
                tb = int(T_B[b])
                g2c = gp.tile([P, tb * G2R], F16, tag="g2c")
                for k in range(tb):
                    nc.gpsimd.indirect_dma_start(
                        out=g2c[:, k * G2R:(k + 1) * G2R], out_offset=None,
                        in_=g2full[:, :],
                        in_offset=bass.IndirectOffsetOnAxis(
                            ap=si2[:, t0 + k:t0 + k + 1], axis=0))
                ohc = ohp.tile([P, tb * P], F16, tag="ohc")
                nc.vector.tensor_tensor(
                    out=_ap(ohc, 0, [[P, tb], [1, P]]),
                    in0=_ap(iota_sb, 0, [[0, tb], [1, P]]),
                    in1=_ap(dst_sb, t0, [[1, tb], [0, P]]),
                    op=ALU.is_equal)
                adps = psB.tile([P, tb], F32, tag="psB")
                for k in range(tb):
                    ohT_ps = psT.tile([P, P], F16, tag="psT")
                    nc.tensor.transpose(out=ohT_ps[:], in_=ohc[:, k * P:(k + 1) * P],
                                        identity=ident_sb[:])
                    ohT = ohtp.tile([P, P], F16, tag="ohT")
                    nc.vector.tensor_copy(out=ohT[:], in_=ohT_ps[:])
                    nc.tensor.matmul(out=adps[:, k:k + 1], lhsT=ohT[:],
                                     rhs=ad2own[:, b:b + 1], start=True, stop=True)
                ech = ep.tile([P, tb], F32, tag="ech")
                nc.vector.tensor_tensor(
                    out=ech[:],
                    in0=_ap(g2c, 0, [[G2R, tb]]),
                    in1=adps[:],
                    op=ALU.add)
                lrch = ep.tile([P, tb], F32, tag="lrch")
                nc.vector.scalar_tensor_tensor(out=lrch[:], in0=ech[:], scalar=0.2,
                                               in1=ech[:], op0=ALU.mult, op1=ALU.max)
                pch = ep.tile([P, tb], F32, tag="pch")
                nc.scalar.activation(pch[:], lrch[:], ACTF.Exp, bias=zcol[:, 0:1])
                vc = vp.tile([P, tb * G2R], F16, tag="vc")
                nc.vector.tensor_copy(out=_ap(vc, 0, [[G2R, tb]]), in_=pch[:])
                for k in range(tb):
                    nc.vector.tensor_scalar_mul(
                        out=vc[:, k * G2R + 1:(k + 1) * G2R],
                        in0=g2c[:, k * G2R + 1:(k + 1) * G2R],
                        scalar1=pch[:, k:k + 1])
                psagg = psA.tile([P, G2R], F32, tag="psA")
                for k in range(tb):
                    nc.tensor.matmul(out=psagg[:], lhsT=ohc[:, k * P:(k + 1) * P],
                                     rhs=vc[:, k * G2R:(k + 1) * G2R],
                                     start=(k == 0), stop=(k == tb - 1))
                es = ep.tile([P, 1], F32, tag="es")
                nc.vector.tensor_tensor(out=es[:], in0=as2own[:, b:b + 1],
                                        in1=ad2own[:, b:b + 1], op=ALU.add)
                lrs = ep.tile([P, 1], F32, tag="lrs")
                nc.vector.scalar_tensor_tensor(out=lrs[:], in0=es[:], scalar=0.2,
                                               in1=es[:], op0=ALU.mult, op1=ALU.max)
                psf = ep.tile([P, 1], F32, tag="psf")
                nc.scalar.activation(psf[:], lrs[:], ACTF.Exp, bias=zcol[:, 0:1])
                st_ = ep.tile([P, 1], F32, tag="st_")
                nc.vector.tensor_tensor(out=st_[:], in0=psagg[:, 0:1], in1=psf[:], op=ALU.add)
                ssb = ep.tile([P, 1], F32, tag="ssb")
                nc.vector.tensor_scalar_add(out=ssb[:], in0=st_[:], scalar1=1e-16)
                sinv = ep.tile([P, 1], F32, tag="sinv")
                nc.vector.reciprocal(out=sinv[:], in_=ssb[:])
                hw2 = ep.tile([P, NC], F32, tag="hw")
                nc.vector.tensor_scalar_mul(out=hw2[:], in0=h2own[:, b * NC:(b + 1) * NC],
                                            scalar1=psf[:, 0:1])
                wf2 = ep.tile([P, NC], F32, tag="wf")
                nc.vector.tensor_tensor(out=wf2[:], in0=psagg[:, 1:1 + NC], in1=hw2[:], op=ALU.add)
                lg = ep.tile([P, NC], F32, tag="t1")
                nc.vector.tensor_scalar_mul(out=lg[:], in0=wf2[:], scalar1=sinv[:, 0:1])
                mx = ep.tile([P, 1], F32, tag="mx")
                nc.vector.reduce_max(mx[:], lg[:], axis=AX.X)
                sh = ep.tile([P, NC], F32, tag="t2")
                nc.vector.tensor_scalar_sub(out=sh[:], in0=lg[:], scalar1=mx[:, 0:1])
                ex = ep.tile([P, NC], F32, tag="t3")
                nc.scalar.activation(ex[:], sh[:], ACTF.Exp, bias=zcol[:, 0:1])
                sm = ep.tile([P, 1], F32, tag="sm")
                nc.vector.reduce_sum(sm[:], ex[:], axis=AX.X)
                ls = ep.tile([P, 1], F32, tag="ls")
                nc.scalar.activation(ls[:], sm[:], ACTF.Ln, bias=zcol[:, 0:1])
                ob = ep.tile([P, NC], F32, tag="t4")
                nc.vector.tensor_scalar_sub(out=ob[:], in0=sh[:], scalar1=ls[:, 0:1])
                nc.sync.dma_start(out=out[b * P:(b + 1) * P, :], in_=ob[:])
                t0 += tb

    nc.compile()
    return nc


def _prep(x, edge_src, edge_dst, W1, a1_src, a1_dst, b1, W2, a2_src, a2_dst, b2):
    """Host-side integer preprocessing (graph partitioning) + param folding."""
    N, F = x.shape
    H, C = a1_src.shape
    NC = W2.shape[1]
    HC = H * C
    NOWN = N // CORES
    NBLK = math.ceil(NOWN / P)
    OWNPAD = NBLK * P
    NFOR = N - NOWN
    FBLK = math.ceil(NFOR / P)
    NODE_BLKS = NBLK + FBLK
    NTAB = NODE_BLKS * P

    # self-loop edges are handled analytically in the block epilogues
    src_all = edge_src
    dst_all = edge_dst

    # per (core, block) edge lists
    core_of = dst_all // NOWN
    per_core = []
    cnt = np.zeros((CORES, NBLK), np.int64)
    for c in range(CORES):
        m = core_of == c
        s, d = src_all[m], dst_all[m] - c * NOWN
        blk = d // P
        order = np.argsort(blk, kind='stable')
        s, d, blk = s[order], d[order], blk[order]
        cnt[c] = np.bincount(blk, minlength=NBLK)
        per_core.append((s, d, blk))
    T_B = np.maximum(1, np.ceil(cnt.max(axis=0) / P).astype(np.int64))
    NT = int(T_B.sum())
    toff = np.concatenate([[0], np.cumsum(T_B)])

    # param folding
    W1r = W1.reshape(F, H, C)
    wsrc = (W1r * a1_src[None]).sum(-1)          # [F, H]
    wdst = (W1r * a1_dst[None]).sum(-1)          # [F, H]
    w1aug = np.concatenate([wsrc, W1, wdst], axis=1).astype(np.float16)   # [F, 8+HC+8]
    b1aug = np.zeros((1, 8 + HC + 8), np.float16)
    b1aug[0, 8:8 + HC] = b1.astype(np.float16)
    G2W = 1 + NC + 1 + 6
    W2K = HC + 1
    w2aug = np.zeros((W2K, G2W), np.float16)
    w2aug[0:HC, 0] = (W2 @ a2_src[0]).astype(np.float16)
    w2aug[0:HC, 1:1 + NC] = W2.astype(np.float16)
    w2aug[0:HC, 1 + NC] = (W2 @ a2_dst[0]).astype(np.float16)
    w2aug[HC, 1:1 + NC] = b2.astype(np.float16)
    ones1 = np.ones((1, P), np.float16)
    iotaf = np.tile(np.arange(P, dtype=np.float32)[None, :], (P, 1))
    ident = np.eye(P, dtype=np.float16)

    xT = np.ascontiguousarray(x.T)               # [F, N] float32

    in_maps = []
    for c in range(CORES):
        own_lo, own_hi = c * NOWN, (c + 1) * NOWN
        # perm: table position -> node
        xTp = np.zeros((F, NTAB), np.float16)
        xTp[:, 0:NOWN] = xT[:, own_lo:own_hi].astype(np.float16)
        fore = np.concatenate([np.arange(0, own_lo), np.arange(own_hi, N)])
        xTp[:, OWNPAD:OWNPAD + NFOR] = xT[:, fore].astype(np.float16)
        # node -> table position
        pos = np.empty(N, np.int64)
        pos[own_lo:own_hi] = np.arange(NOWN)
        pos[fore] = OWNPAD + np.arange(NFOR)

        s, d, blk = per_core[c]
        sidx1 = np.zeros((P, NT), np.int32)
        sidx2 = np.zeros((P, NT), np.int32)
        dstc = np.full((P, NT), -1.0, np.float32)
        bstart = np.concatenate([[0], np.cumsum(np.bincount(blk, minlength=NBLK))])
        for b in range(NBLK):
            eb = slice(bstart[b], bstart[b + 1])
            sb_, db_ = s[eb], d[eb]
            n = len(sb_)
            for k in range(int(T_B[b])):
                lo, hi = k * P, min((k + 1) * P, n)
                if lo >= n:
                    break
                t = toff[b] + k
                m = hi - lo
                sidx1[0:m, t] = pos[sb_[lo:hi]]
                sidx2[0:m, t] = (sb_[lo:hi] // NOWN) * OWNPAD + (sb_[lo:hi] % NOWN)
                dstc[0:m, t] = (db_[lo:hi] % P).astype(np.float32)
        in_maps.append({
            "xT": xTp, "w1aug": w1aug, "b1aug": b1aug, "w2aug": w2aug,
            "ones1": ones1, "iotaf": iotaf.astype(np.float32), "ident": ident,
            "sidx1": sidx1, "sidx2": sidx2, "dstc": dstc,
        })
    meta = dict(N=N, F=F, H=H, C=C, NC=NC, T_B=T_B, NTAB=NTAB, NBLK=NBLK,
                NODE_BLKS=NODE_BLKS, NOWN=NOWN)
    return in_maps, meta


_CACHED = {}


def run(inputs, eshift=-4.0, trace=False):
    in_maps, meta = _prep(**inputs)
    key = (meta["N"], meta["F"], meta["NC"], tuple(meta["T_B"]))
    if key not in _CACHED:
        _CACHED[key] = _build_program(meta["N"], meta["F"], meta["H"], meta["C"],
                                      meta["NC"], meta["T_B"], meta["NTAB"],
                                      meta["NBLK"], meta["NODE_BLKS"], eshift)
    nc = _CACHED[key]
    res = bass_utils.run_bass_kernel_spmd(nc, in_maps,
                                          core_ids=list(range(CORES)),
                                          trace=trace)
    outs = [res.results[c]["out"][:meta["NOWN"]] for c in range(CORES)]
    full = np.concatenate(outs, axis=0).astype(np.float32)
    return full, res


def kernel(**inputs):
    full, _ = run(inputs)
    return full


# revision 5
# speedup vs baseline: 1.0504x; 1.0039x over previous
"""2-layer GAT (PyG-style GATConv x2 + log_softmax) on 8 Trainium2 NeuronCores.

Sharding: dst-node sharding (each core owns N/8 destination nodes and all
edges into them). Node features (x) are replicated; each core computes the
full layer-1 node transform, so the only cross-core exchange is one
AllGather of the small layer-2 per-node table between layers.

Edge phase per core: edges sorted by dst block (128 dst nodes per block),
tiles of 128 edges. Per tile: one indirect DMA gathers the [as1|h] rows of
the edge sources from a DRAM table; ad1[dst] is reconstructed on-chip with
a one-hot matmul (no second gather); attention weights p = exp(lrelu(as+ad))
are computed chunked per block; a one-hot aggregation matmul accumulates
[p | p*h] into the per-block PSUM, which is then normalized (segment
softmax) without materializing per-edge alphas.
"""
import sys
sys.path.insert(0, '/opt/trn_rl_repo')
if '/root/.axon_site' not in sys.path:
    sys.path.insert(0, '/root/.axon_site')

import math
import numpy as np

import concourse.bass as bass
import concourse.bacc as bacc
import concourse.tile as tile
from concourse import mybir
from concourse import bass_utils

F16 = mybir.dt.float16
F32 = mybir.dt.float32
I32 = mybir.dt.int32
AX = mybir.AxisListType
ALU = mybir.AluOpType
ACTF = mybir.ActivationFunctionType

CORES = 8
P = 128


def _ap(t, off, dims):
    """AP over pool tile t: partition dim from the tile + given free dims."""
    base = t[:]
    return bass.AP(base.tensor, base.offset + off, [list(base.ap[0])] + [list(d) for d in dims])


def _build_program(N, F, H, C, NC, T_B, NTAB, NBLK, NODE_BLKS, ESHIFT):
    """Build the SPMD Bass program (identical across cores)."""
    HC = H * C
    OWNPAD = NBLK * P
    NT = int(sum(T_B))
    G1W = 8 + HC            # [as1 | h] row width (72)
    G2W = 1 + NC + 1 + 6    # [as2 | h2 | ad2 | pad] = 48
    G2R = 1 + NC            # gathered part of a g2 row (41)
    W2K = HC + 1            # 65

    nc = bacc.Bacc("TRN2", target_bir_lowering=False, debug=False,
                   num_devices=CORES)

    xT = nc.dram_tensor("xT", [F, NODE_BLKS * P], F16, kind="ExternalInput").ap()
    w1aug = nc.dram_tensor("w1aug", [F, 8 + HC + 8], F16, kind="ExternalInput").ap()
    b1aug = nc.dram_tensor("b1aug", [1, 8 + HC + 8], F16, kind="ExternalInput").ap()
    w2aug = nc.dram_tensor("w2aug", [W2K, G2W], F16, kind="ExternalInput").ap()
    ones1 = nc.dram_tensor("ones1", [1, P], F16, kind="ExternalInput").ap()
    iotaf = nc.dram_tensor("iotaf", [P, P], F32, kind="ExternalInput").ap()
    ident = nc.dram_tensor("ident", [P, P], F16, kind="ExternalInput").ap()
    sidx1 = nc.dram_tensor("sidx1", [P, NT], I32, kind="ExternalInput").ap()
    sidx2 = nc.dram_tensor("sidx2", [P, NT], I32, kind="ExternalInput").ap()
    dstc = nc.dram_tensor("dstc", [P, NT], F32, kind="ExternalInput").ap()
    dstrow = nc.dram_tensor("dstrow", [1, NT * P], F16, kind="ExternalInput").ap()
    iotac = nc.dram_tensor("iotac", [P, 1], F32, kind="ExternalInput").ap()
    out = nc.dram_tensor("out", [OWNPAD, NC], F32, kind="ExternalOutput").ap()

    with tile.TileContext(nc) as tc:
        with tc.tile_pool(name="const", bufs=1) as cp, \
             tc.tile_pool(name="xp", bufs=4) as xp, \
             tc.tile_pool(name="stp", bufs=3) as stp, \
             tc.tile_pool(name="gp", bufs=2) as gp, \
             tc.tile_pool(name="ohp", bufs=2) as ohp, \
             tc.tile_pool(name="vp", bufs=2) as vp, \
             tc.tile_pool(name="ohtp", bufs=3) as ohtp, \
             tc.tile_pool(name="ep", bufs=2) as ep, \
             tc.tile_pool(name="psA", bufs=2, space="PSUM") as psA, \
             tc.tile_pool(name="psB", bufs=2, space="PSUM") as psB, \
             tc.tile_pool(name="psT", bufs=2, space="PSUM") as psT, \
             tc.tile_pool(name="psW", bufs=2, space="PSUM") as psW, \
             tc.tile_pool(name="dram", bufs=1, space="DRAM") as dp:

            g1tab = dp.tile([NTAB, G1W], F16)
            g2own = dp.tile([OWNPAD, G2W], F16)
            g2full = dp.tile([CORES * OWNPAD, G2W], F16, addr_space="Shared")

            # ---- resident constants -------------------------------------
            iota_sb = cp.tile([P, P], F32)
            nc.sync.dma_start(out=iota_sb[:], in_=iotaf[:, :])
            iotac_sb = cp.tile([P, 1], F32)
            nc.sync.dma_start(out=iotac_sb[:], in_=iotac[:, :])
            ident_sb = cp.tile([P, P], F16)
            nc.sync.dma_start(out=ident_sb[:], in_=ident[:, :])
            w1a0 = cp.tile([P, 8 + HC + 8], F16)
            nc.sync.dma_start(out=w1a0[:], in_=w1aug[0:P, :])
            w1a1 = cp.tile([P, 8 + HC + 8], F16)
            nc.sync.dma_start(out=w1a1[:], in_=w1aug[P:2 * P, :])
            b1a = cp.tile([1, 8 + HC + 8], F16)
            nc.sync.dma_start(out=b1a[:], in_=b1aug[:, :])
            w2a = cp.tile([W2K, G2W], F16)
            nc.sync.dma_start(out=w2a[:], in_=w2aug[:, :])
            ones_sb = cp.tile([1, P], F16)
            nc.sync.dma_start(out=ones_sb[:], in_=ones1[:, :])
            si1 = cp.tile([P, NT], I32)
            nc.sync.dma_start(out=si1[:], in_=sidx1[:, :])
            si2 = cp.tile([P, NT], I32)
            nc.sync.dma_start(out=si2[:], in_=sidx2[:, :])
            dst_sb = cp.tile([P, NT], F32)
            nc.sync.dma_start(out=dst_sb[:], in_=dstc[:, :])
            ad1own = cp.tile([P, NBLK * 8], F16)
            ad2own = cp.tile([P, NBLK], F16)
            as1own = cp.tile([P, NBLK * 8], F16)
            h1own = cp.tile([P, NBLK * HC], F16)
            as2own = cp.tile([P, NBLK], F16)
            h2own = cp.tile([P, NBLK * NC], F16)
            lhsT65 = cp.tile([W2K, P], F16)
            nc.vector.memset(lhsT65[:], 0.0)
            nc.vector.memset(lhsT65[HC:W2K, :], 1.0)
            zcol = cp.tile([P, 1], F32)
            nc.vector.memset(zcol[:], 0.0)
            scol = cp.tile([P, 1], F32)
            nc.vector.memset(scol[:], ESHIFT)

            # ---- node phase: g1 table for every node --------------------
            for b in range(NODE_BLKS):
                x0 = xp.tile([P, P], F16, tag="x0")
                nc.sync.dma_start(out=x0[:], in_=xT[0:P, b * P:(b + 1) * P])
                x1 = xp.tile([P, P], F16, tag="x1")
                nc.sync.dma_start(out=x1[:], in_=xT[P:2 * P, b * P:(b + 1) * P])
                ps = psA.tile([P, 8 + HC + 8], F32, tag="psA")
                nc.tensor.matmul(out=ps[:], lhsT=x0[:], rhs=w1a0[:], start=True, stop=False)
                nc.tensor.matmul(out=ps[:], lhsT=x1[:], rhs=w1a1[:], start=False, stop=False)
                nc.tensor.matmul(out=ps[:], lhsT=ones_sb[:], rhs=b1a[:], start=False, stop=True)
                st = stp.tile([P, G1W], F16, tag="g1st")
                nc.vector.tensor_copy(out=st[:], in_=ps[:, 0:G1W])
                nc.sync.dma_start(out=g1tab[b * P:(b + 1) * P, :], in_=st[:])
                if b < NBLK:
                    nc.vector.tensor_copy(out=ad1own[:, b * 8:(b + 1) * 8],
                                          in_=ps[:, G1W:G1W + 8])
                    nc.vector.tensor_copy(out=as1own[:, b * 8:(b + 1) * 8],
                                          in_=ps[:, 0:8])
                    nc.vector.tensor_copy(out=h1own[:, b * HC:(b + 1) * HC],
                                          in_=ps[:, 8:8 + HC])

            # ---- layer 1 edge phase + epilogue --------------------------
            t0 = 0
            for b in range(NBLK):
                tb = int(T_B[b])
                g1c = gp.tile([P, tb * G1W], F16, tag="g1c")
                for k in range(tb):
                    nc.gpsimd.indirect_dma_start(
                        out=g1c[:, k * G1W:(k + 1) * G1W], out_offset=None,
                        in_=g1tab[:, :],
                        in_offset=bass.IndirectOffsetOnAxis(
                            ap=si1[:, t0 + k:t0 + k + 1], axis=0))
                ohc = ohp.tile([P, tb * P], F16, tag="ohc")
                nc.vector.tensor_tensor(
                    out=_ap(ohc, 0, [[P, tb], [1, P]]),
                    in0=_ap(iota_sb, 0, [[0, tb], [1, P]]),
                    in1=_ap(dst_sb, t0, [[1, tb], [0, P]]),
                    op=ALU.is_equal)
                dsr = stp.tile([1, tb * P], F16, tag="dsr")
                nc.sync.dma_start(out=dsr[:], in_=dstrow[0:1, t0 * P:(t0 + tb) * P])
                ohtc = ohtp.tile([P, tb * P], F16, tag="ohtc")
                for g in range(0, tb, 4):
                    gw = min(4, tb - g)
                    bps = psW.tile([P, 4 * P], F32, tag="psW")
                    nc.tensor.matmul(out=bps[:, 0:gw * P], lhsT=ones_sb[:],
                                     rhs=dsr[0:1, g * P:(g + gw) * P],
                                     start=True, stop=True)
                    nc.vector.tensor_tensor(
                        out=ohtc[:, g * P:(g + gw) * P], in0=bps[:, 0:gw * P],
                        in1=_ap(iotac_sb, 0, [[0, gw * P]]),
                        op=ALU.is_equal)
                adps = psB.tile([P, tb * 8], F32, tag="psB")
                for k in range(tb):
                    nc.tensor.matmul(out=adps[:, k * 8:(k + 1) * 8],
                                     lhsT=ohtc[:, k * P:(k + 1) * P],
                                     rhs=ad1own[:, b * 8:(b + 1) * 8],
                                     start=True, stop=True)
                ech = ep.tile([P, tb * 8], F32, tag="ech")
                nc.vector.tensor_tensor(
                    out=_ap(ech, 0, [[8, tb], [1, 8]]),
                    in0=_ap(g1c, 0, [[G1W, tb], [1, 8]]),
                    in1=_ap(adps, 0, [[8, tb], [1, 8]]),
                    op=ALU.add)
                lrch = ep.tile([P, tb * 8], F32, tag="lrch")
                nc.vector.scalar_tensor_tensor(out=lrch[:], in0=ech[:], scalar=0.2,
                                               in1=ech[:], op0=ALU.mult, op1=ALU.max)
                pch = ep.tile([P, tb * 8], F32, tag="pch")
                nc.scalar.activation(pch[:], lrch[:], ACTF.Exp, bias=scol[:, 0:1])
                vc = vp.tile([P, tb * G1W], F16, tag="vc")
                nc.vector.tensor_copy(
                    out=_ap(vc, 0, [[G1W, tb], [1, 8]]),
                    in_=_ap(pch, 0, [[8, tb], [1, 8]]))
                nc.vector.tensor_tensor(
                    out=_ap(vc, 8, [[G1W, tb], [8, H], [1, C]]),
                    in0=_ap(g1c, 8, [[G1W, tb], [8, H], [1, C]]),
                    in1=_ap(pch, 0, [[8, tb], [1, H], [0, C]]),
                    op=ALU.mult)
                psagg = psA.tile([P, G1W], F32, tag="psA")
                for k in range(tb):
                    nc.tensor.matmul(out=psagg[:], lhsT=ohc[:, k * P:(k + 1) * P],
                                     rhs=vc[:, k * G1W:(k + 1) * G1W],
                                     start=(k == 0), stop=(k == tb - 1))
                # epilogue: self-loop term, segment-softmax normalize, ELU
                es = ep.tile([P, 8], F32, tag="es")
                nc.vector.tensor_tensor(out=es[:], in0=as1own[:, b * 8:(b + 1) * 8],
                                        in1=ad1own[:, b * 8:(b + 1) * 8], op=ALU.add)
                lrs = ep.tile([P, 8], F32, tag="lrs")
                nc.vector.scalar_tensor_tensor(out=lrs[:], in0=es[:], scalar=0.2,
                                               in1=es[:], op0=ALU.mult, op1=ALU.max)
                psf = ep.tile([P, 8], F32, tag="psf")
                nc.scalar.activation(psf[:], lrs[:], ACTF.Exp, bias=scol[:, 0:1])
                st_ = ep.tile([P, 8], F32, tag="st_")
                nc.vector.tensor_tensor(out=st_[:], in0=psagg[:, 0:8], in1=psf[:], op=ALU.add)
                ssb = ep.tile([P, 8], F32, tag="ssb")
                nc.vector.tensor_scalar_add(out=ssb[:], in0=st_[:], scalar1=1e-16)
                sinv = ep.tile([P, 8], F32, tag="sinv")
                nc.vector.reciprocal(out=sinv[:], in_=ssb[:])
                hw = ep.tile([P, HC], F32, tag="hw")
                nc.vector.tensor_tensor(
                    out=_ap(hw, 0, [[C, H], [1, C]]),
                    in0=_ap(h1own, b * HC, [[C, H], [1, C]]),
                    in1=_ap(psf, 0, [[1, H], [0, C]]),
                    op=ALU.mult)
                wf = ep.tile([P, HC], F32, tag="wf")
                nc.vector.tensor_tensor(out=wf[:], in0=psagg[:, 8:8 + HC], in1=hw[:], op=ALU.add)
                h1f = ep.tile([P, HC], F32, tag="h1f")
                nc.vector.tensor_tensor(
                    out=_ap(h1f, 0, [[C, H], [1, C]]),
                    in0=_ap(wf, 0, [[C, H], [1, C]]),
                    in1=_ap(sinv, 0, [[1, H], [0, C]]),
                    op=ALU.mult)
                t1 = ep.tile([P, HC], F32, tag="t1")
                nc.vector.tensor_scalar_min(out=t1[:], in0=h1f[:], scalar1=0.0)
                t2 = ep.tile([P, HC], F32, tag="t2")
                nc.scalar.activation(t2[:], t1[:], ACTF.Exp, bias=zcol[:, 0:1])
                t3 = ep.tile([P, HC], F32, tag="t3")
                nc.vector.tensor_scalar_max(out=t3[:], in0=h1f[:], scalar1=0.0)
                t4 = ep.tile([P, HC], F32, tag="t4")
                nc.vector.tensor_tensor(out=t4[:], in0=t2[:], in1=t3[:], op=ALU.add)
                h1e = ep.tile([P, HC], F16, tag="h1e")
                nc.vector.tensor_scalar_add(out=h1e[:], in0=t4[:], scalar1=-1.0)
                trp = psT.tile([HC, P], F16, tag="psT")
                nc.tensor.transpose(out=trp[:], in_=h1e[:], identity=ident_sb[:])
                nc.vector.tensor_copy(out=lhsT65[0:HC, :], in_=trp[:])
                ps2 = psB.tile([P, G2W], F32, tag="psB")
                nc.tensor.matmul(out=ps2[:], lhsT=lhsT65[:], rhs=w2a[:],
                                 start=True, stop=True)
                g2st = stp.tile([P, G2W], F16, tag="g2st")
                nc.vector.tensor_copy(out=g2st[:], in_=ps2[:])
                nc.sync.dma_start(out=g2own[b * P:(b + 1) * P, :], in_=g2st[:])
                nc.vector.tensor_copy(out=ad2own[:, b:b + 1], in_=ps2[:, G2R:G2R + 1])
                nc.vector.tensor_copy(out=as2own[:, b:b + 1], in_=ps2[:, 0:1])
                nc.vector.tensor_copy(out=h2own[:, b * NC:(b + 1) * NC], in_=ps2[:, 1:1 + NC])
                t0 += tb

            # ---- halo exchange of layer-2 node table --------------------
            nc.gpsimd.collective_compute(
                "AllGather", ALU.bypass,
                ins=[g2own[:].opt()], outs=[g2full[:].opt()],
                replica_groups=[list(range(CORES))])

            # ---- layer 2 edge phase + epilogue --------------------------
            t0 = 0
            for b in range(NBLK):
                tb = int(T_B[b])
                g2c = gp.tile([P, tb * G2R], F16, tag="g2c")
                for k in range(tb):
                    nc.gpsimd.indirect_dma_start(
                        out=g2c[:, k * G2R:(k + 1) * G2R], out_offset=None,
                        in_=g2full[:, :],
                        in_offset=bass.IndirectOffsetOnAxis(
                            ap=si2[:, t0 + k:t0 + k + 1], axis=0))
                ohc = ohp.tile([P, tb * P], F16, tag="ohc")
                nc.vector.tensor_tensor(
                    out=_ap(ohc, 0, [[P, tb], [1, P]]),
                    in0=_ap(iota_sb, 0, [[0, tb], [1, P]]),
                    in1=_ap(dst_sb, t0, [[1, tb], [0, P]]),
                    op=ALU.is_equal)
                dsr = stp.tile([1, tb * P], F16, tag="dsr")
                nc.sync.dma_start(out=dsr[:], in_=dstrow[0:1, t0 * P:(t0 + tb) * P])
                ohtc = ohtp.tile([P, tb * P], F16, tag="ohtc")
                for g in range(0, tb, 4):
                    gw = min(4, tb - g)
                    bps = psW.tile([P, 4 * P], F32, tag="psW")
                    nc.tensor.matmul(out=bps[:, 0:gw * P], lhsT=ones_sb[:],
                                     rhs=dsr[0:1, g * P:(g + gw) * P],
                                     start=True, stop=True)
                    nc.vector.tensor_tensor(
                        out=ohtc[:, g * P:(g + gw) * P], in0=bps[:, 0:gw * P],
                        in1=_ap(iotac_sb, 0, [[0, gw * P]]),
                        op=ALU.is_equal)
                adps = psB.tile([P, tb], F32, tag="psB")
                for k in range(tb):
                    nc.tensor.matmul(out=adps[:, k:k + 1],
                                     lhsT=ohtc[:, k * P:(k + 1) * P],
                                     rhs=ad2own[:, b:b + 1], start=True, stop=True)
                ech = ep.tile([P, tb], F32, tag="ech")
                nc.vector.tensor_tensor(
                    out=ech[:],
                    in0=_ap(g2c, 0, [[G2R, tb]]),
                    in1=adps[:],
                    op=ALU.add)
                lrch = ep.tile([P, tb], F32, tag="lrch")
                nc.vector.scalar_tensor_tensor(out=lrch[:], in0=ech[:], scalar=0.2,
                                               in1=ech[:], op0=ALU.mult, op1=ALU.max)
                pch = ep.tile([P, tb], F32, tag="pch")
                nc.scalar.activation(pch[:], lrch[:], ACTF.Exp, bias=zcol[:, 0:1])
                vc = vp.tile([P, tb * G2R], F16, tag="vc")
                nc.vector.tensor_copy(out=_ap(vc, 0, [[G2R, tb]]), in_=pch[:])
                for k in range(tb):
                    nc.vector.tensor_scalar_mul(
                        out=vc[:, k * G2R + 1:(k + 1) * G2R],
                        in0=g2c[:, k * G2R + 1:(k + 1) * G2R],
                        scalar1=pch[:, k:k + 1])
                psagg = psA.tile([P, G2R], F32, tag="psA")
                for k in range(tb):
                    nc.tensor.matmul(out=psagg[:], lhsT=ohc[:, k * P:(k + 1) * P],
                                     rhs=vc[:, k * G2R:(k + 1) * G2R],
                                     start=(k == 0), stop=(k == tb - 1))
                es = ep.tile([P, 1], F32, tag="es")
                nc.vector.tensor_tensor(out=es[:], in0=as2own[:, b:b + 1],
                                        in1=ad2own[:, b:b + 1], op=ALU.add)
                lrs = ep.tile([P, 1], F32, tag="lrs")
                nc.vector.scalar_tensor_tensor(out=lrs[:], in0=es[:], scalar=0.2,
                                               in1=es[:], op0=ALU.mult, op1=ALU.max)
                psf = ep.tile([P, 1], F32, tag="psf")
                nc.scalar.activation(psf[:], lrs[:], ACTF.Exp, bias=zcol[:, 0:1])
                st_ = ep.tile([P, 1], F32, tag="st_")
                nc.vector.tensor_tensor(out=st_[:], in0=psagg[:, 0:1], in1=psf[:], op=ALU.add)
                ssb = ep.tile([P, 1], F32, tag="ssb")
                nc.vector.tensor_scalar_add(out=ssb[:], in0=st_[:], scalar1=1e-16)
                sinv = ep.tile([P, 1], F32, tag="sinv")
                nc.vector.reciprocal(out=sinv[:], in_=ssb[:])
                hw2 = ep.tile([P, NC], F32, tag="hw")
                nc.vector.tensor_scalar_mul(out=hw2[:], in0=h2own[:, b * NC:(b + 1) * NC],
                                            scalar1=psf[:, 0:1])
                wf2 = ep.tile([P, NC], F32, tag="wf")
                nc.vector.tensor_tensor(out=wf2[:], in0=psagg[:, 1:1 + NC], in1=hw2[:], op=ALU.add)
                lg = ep.tile([P, NC], F32, tag="t1")
                nc.vector.tensor_scalar_mul(out=lg[:], in0=wf2[:], scalar1=sinv[:, 0:1])
                mx = ep.tile([P, 1], F32, tag="mx")
                nc.vector.reduce_max(mx[:], lg[:], axis=AX.X)
                sh = ep.tile([P, NC], F32, tag="t2")
                nc.vector.tensor_scalar_sub(out=sh[:], in0=lg[:], scalar1=mx[:, 0:1])
                ex = ep.tile([P, NC], F32, tag="t3")
                nc.scalar.activation(ex[:], sh[:], ACTF.Exp, bias=zcol[:, 0:1])
                sm = ep.tile([P, 1], F32, tag="sm")
                nc.vector.reduce_sum(sm[:], ex[:], axis=AX.X)
                ls = ep.tile([P, 1], F32, tag="ls")
                nc.scalar.activation(ls[:], sm[:], ACTF.Ln, bias=zcol[:, 0:1])
                ob = ep.tile([P, NC], F32, tag="t4")
                nc.vector.tensor_scalar_sub(out=ob[:], in0=sh[:], scalar1=ls[:, 0:1])
                nc.sync.dma_start(out=out[b * P:(b + 1) * P, :], in_=ob[:])
                t0 += tb

    nc.compile()
    return nc


def _prep(x, edge_src, edge_dst, W1, a1_src, a1_dst, b1, W2, a2_src, a2_dst, b2):
    """Host-side integer preprocessing (graph partitioning) + param folding."""
    N, F = x.shape
    H, C = a1_src.shape
    NC = W2.shape[1]
    HC = H * C
    NOWN = N // CORES
    NBLK = math.ceil(NOWN / P)
    OWNPAD = NBLK * P
    NFOR = N - NOWN
    FBLK = math.ceil(NFOR / P)
    NODE_BLKS = NBLK + FBLK
    NTAB = NODE_BLKS * P

    # self-loop edges are handled analytically in the block epilogues
    src_all = edge_src
    dst_all = edge_dst

    # per (core, block) edge lists
    core_of = dst_all // NOWN
    per_core = []
    cnt = np.zeros((CORES, NBLK), np.int64)
    for c in range(CORES):
        m = core_of == c
        s, d = src_all[m], dst_all[m] - c * NOWN
        blk = d // P
        order = np.argsort(blk, kind='stable')
        s, d, blk = s[order], d[order], blk[order]
        cnt[c] = np.bincount(blk, minlength=NBLK)
        per_core.append((s, d, blk))
    T_B = np.maximum(1, np.ceil(cnt.max(axis=0) / P).astype(np.int64))
    NT = int(T_B.sum())
    toff = np.concatenate([[0], np.cumsum(T_B)])

    # param folding
    W1r = W1.reshape(F, H, C)
    wsrc = (W1r * a1_src[None]).sum(-1)          # [F, H]
    wdst = (W1r * a1_dst[None]).sum(-1)          # [F, H]
    w1aug = np.concatenate([wsrc, W1, wdst], axis=1).astype(np.float16)   # [F, 8+HC+8]
    b1aug = np.zeros((1, 8 + HC + 8), np.float16)
    b1aug[0, 8:8 + HC] = b1.astype(np.float16)
    G2W = 1 + NC + 1 + 6
    W2K = HC + 1
    w2aug = np.zeros((W2K, G2W), np.float16)
    w2aug[0:HC, 0] = (W2 @ a2_src[0]).astype(np.float16)
    w2aug[0:HC, 1:1 + NC] = W2.astype(np.float16)
    w2aug[0:HC, 1 + NC] = (W2 @ a2_dst[0]).astype(np.float16)
    w2aug[HC, 1:1 + NC] = b2.astype(np.float16)
    ones1 = np.ones((1, P), np.float16)
    iotaf = np.tile(np.arange(P, dtype=np.float32)[None, :], (P, 1))
    ident = np.eye(P, dtype=np.float16)

    xT = np.ascontiguousarray(x.T)               # [F, N] float32

    in_maps = []
    for c in range(CORES):
        own_lo, own_hi = c * NOWN, (c + 1) * NOWN
        # perm: table position -> node
        xTp = np.zeros((F, NTAB), np.float16)
        xTp[:, 0:NOWN] = xT[:, own_lo:own_hi].astype(np.float16)
        fore = np.concatenate([np.arange(0, own_lo), np.arange(own_hi, N)])
        xTp[:, OWNPAD:OWNPAD + NFOR] = xT[:, fore].astype(np.float16)
        # node -> table position
        pos = np.empty(N, np.int64)
        pos[own_lo:own_hi] = np.arange(NOWN)
        pos[fore] = OWNPAD + np.arange(NFOR)

        s, d, blk = per_core[c]
        sidx1 = np.zeros((P, NT), np.int32)
        sidx2 = np.zeros((P, NT), np.int32)
        dstc = np.full((P, NT), -1.0, np.float32)
        bstart = np.concatenate([[0], np.cumsum(np.bincount(blk, minlength=NBLK))])
        for b in range(NBLK):
            eb = slice(bstart[b], bstart[b + 1])
            sb_, db_ = s[eb], d[eb]
            n = len(sb_)
            for k in range(int(T_B[b])):
                lo, hi = k * P, min((k + 1) * P, n)
                if lo >= n:
                    break
                t = toff[b] + k
                m = hi - lo
                sidx1[0:m, t] = pos[sb_[lo:hi]]
                sidx2[0:m, t] = (sb_[lo:hi] // NOWN) * OWNPAD + (sb_[lo:hi] % NOWN)
                dstc[0:m, t] = (db_[lo:hi] % P).astype(np.float32)
        in_maps.append({
            "xT": xTp, "w1aug": w1aug, "b1aug": b1aug, "w2aug": w2aug,
            "ones1": ones1, "iotaf": iotaf.astype(np.float32), "ident": ident,
            "sidx1": sidx1, "sidx2": sidx2, "dstc": dstc,
            "dstrow": dstc.T.reshape(1, NT * P).astype(np.float16),
            "iotac": np.arange(P, dtype=np.float32)[:, None],
        })
    meta = dict(N=N, F=F, H=H, C=C, NC=NC, T_B=T_B, NTAB=NTAB, NBLK=NBLK,
                NODE_BLKS=NODE_BLKS, NOWN=NOWN)
    return in_maps, meta


_CACHED = {}


def run(inputs, eshift=-4.0, trace=False, tmpdir=None):
    in_maps, meta = _prep(**inputs)
    key = (meta["N"], meta["F"], meta["NC"], tuple(meta["T_B"]))
    if key not in _CACHED:
        _CACHED[key] = _build_program(meta["N"], meta["F"], meta["H"], meta["C"],
                                      meta["NC"], meta["T_B"], meta["NTAB"],
                                      meta["NBLK"], meta["NODE_BLKS"], eshift)
    nc = _CACHED[key]
    kw = {"tmpdir": tmpdir} if tmpdir else {}
    res = bass_utils.run_bass_kernel_spmd(nc, in_maps,
                                          core_ids=list(range(CORES)),
                                          trace=trace, **kw)
    outs = [res.results[c]["out"][:meta["NOWN"]] for c in range(CORES)]
    full = np.concatenate(outs, axis=0).astype(np.float32)
    return full, res


def kernel(**inputs):
    full, _ = run(inputs)
    return full


# revision 6
# speedup vs baseline: 1.1755x; 1.1191x over previous
"""2-layer GAT (PyG-style GATConv x2 + log_softmax) on 8 Trainium2 NeuronCores.

Sharding: dst-node sharding (each core owns N/8 destination nodes and all
edges into them). Node features (x) are replicated; each core computes the
full layer-1 node transform, so the only cross-core exchange is one
AllGather of the small layer-2 per-node table between layers.

Edge phase per core: edges sorted by dst block (128 dst nodes per block),
tiles of 128 edges. Per tile: one indirect DMA gathers the [as1|h] rows of
the edge sources from a DRAM table; ad1[dst] is reconstructed on-chip with
a one-hot matmul (no second gather); attention weights p = exp(lrelu(as+ad))
are computed chunked per block; a one-hot aggregation matmul accumulates
[p | p*h] into the per-block PSUM, which is then normalized (segment
softmax) without materializing per-edge alphas.
"""
import sys
sys.path.insert(0, '/opt/trn_rl_repo')
if '/root/.axon_site' not in sys.path:
    sys.path.insert(0, '/root/.axon_site')

import math
import numpy as np

import concourse.bass as bass
import concourse.bacc as bacc
import concourse.tile as tile
from concourse import mybir
from concourse import bass_utils

F16 = mybir.dt.float16
F32 = mybir.dt.float32
I32 = mybir.dt.int32
AX = mybir.AxisListType
ALU = mybir.AluOpType
ACTF = mybir.ActivationFunctionType

CORES = 8
P = 128


def _ap(t, off, dims):
    """AP over pool tile t: partition dim from the tile + given free dims."""
    base = t[:]
    return bass.AP(base.tensor, base.offset + off, [list(base.ap[0])] + [list(d) for d in dims])


def _build_program(N, F, H, C, NC, T_B, NTAB, NBLK, NODE_BLKS, ESHIFT):
    """Build the SPMD Bass program (identical across cores)."""
    HC = H * C
    OWNPAD = NBLK * P
    NT = int(sum(T_B))
    G1W = 8 + HC            # [as1 | h] row width (72)
    G2W = 1 + NC + 1 + 6    # [as2 | h2 | ad2 | pad] = 48
    G2R = 1 + NC            # gathered part of a g2 row (41)
    W2K = HC + 1            # 65

    nc = bacc.Bacc("TRN2", target_bir_lowering=False, debug=False,
                   num_devices=CORES)

    xT = nc.dram_tensor("xT", [F, NODE_BLKS * P], F16, kind="ExternalInput").ap()
    w1aug = nc.dram_tensor("w1aug", [F, 8 + HC + 8], F16, kind="ExternalInput").ap()
    b1aug = nc.dram_tensor("b1aug", [1, 8 + HC + 8], F16, kind="ExternalInput").ap()
    w2aug = nc.dram_tensor("w2aug", [W2K, G2W], F16, kind="ExternalInput").ap()
    ones1 = nc.dram_tensor("ones1", [1, P], F16, kind="ExternalInput").ap()
    iotaf = nc.dram_tensor("iotaf", [P, P], F32, kind="ExternalInput").ap()
    ident = nc.dram_tensor("ident", [P, P], F16, kind="ExternalInput").ap()
    sidx1 = nc.dram_tensor("sidx1", [P, NT], I32, kind="ExternalInput").ap()
    sidx2 = nc.dram_tensor("sidx2", [P, NT], I32, kind="ExternalInput").ap()
    dstc = nc.dram_tensor("dstc", [P, NT], F32, kind="ExternalInput").ap()
    dstrow = nc.dram_tensor("dstrow", [1, NT * P], F16, kind="ExternalInput").ap()
    iotac = nc.dram_tensor("iotac", [P, 1], F32, kind="ExternalInput").ap()
    out = nc.dram_tensor("out", [OWNPAD, NC], F32, kind="ExternalOutput").ap()

    with tile.TileContext(nc) as tc:
        with tc.tile_pool(name="const", bufs=1) as cp, \
             tc.tile_pool(name="xp", bufs=4) as xp, \
             tc.tile_pool(name="stp", bufs=3) as stp, \
             tc.tile_pool(name="gp", bufs=3) as gp, \
             tc.tile_pool(name="ohp", bufs=2) as ohp, \
             tc.tile_pool(name="vp", bufs=2) as vp, \
             tc.tile_pool(name="ohtp", bufs=2) as ohtp, \
             tc.tile_pool(name="ep", bufs=2) as ep, \
             tc.tile_pool(name="psA", bufs=2, space="PSUM") as psA, \
             tc.tile_pool(name="psB", bufs=2, space="PSUM") as psB, \
             tc.tile_pool(name="psT", bufs=2, space="PSUM") as psT, \
             tc.tile_pool(name="psW", bufs=2, space="PSUM") as psW, \
             tc.tile_pool(name="dram", bufs=1, space="DRAM") as dp:

            g1tab = dp.tile([NTAB, G1W], F16)
            g2own = dp.tile([OWNPAD, G2W], F16)
            g2full = dp.tile([CORES * OWNPAD, G2W], F16, addr_space="Shared")

            # ---- resident constants -------------------------------------
            iota_sb = cp.tile([P, P], F32)
            nc.sync.dma_start(out=iota_sb[:], in_=iotaf[:, :])
            iotac_sb = cp.tile([P, 1], F32)
            nc.sync.dma_start(out=iotac_sb[:], in_=iotac[:, :])
            ident_sb = cp.tile([P, P], F16)
            nc.sync.dma_start(out=ident_sb[:], in_=ident[:, :])
            w1a0 = cp.tile([P, 8 + HC + 8], F16)
            nc.sync.dma_start(out=w1a0[:], in_=w1aug[0:P, :])
            w1a1 = cp.tile([P, 8 + HC + 8], F16)
            nc.sync.dma_start(out=w1a1[:], in_=w1aug[P:2 * P, :])
            b1a = cp.tile([1, 8 + HC + 8], F16)
            nc.sync.dma_start(out=b1a[:], in_=b1aug[:, :])
            w2a = cp.tile([W2K, G2W], F16)
            nc.sync.dma_start(out=w2a[:], in_=w2aug[:, :])
            ones_sb = cp.tile([1, P], F16)
            nc.sync.dma_start(out=ones_sb[:], in_=ones1[:, :])
            si1 = cp.tile([P, NT], I32)
            nc.sync.dma_start(out=si1[:], in_=sidx1[:, :])
            si2 = cp.tile([P, NT], I32)
            nc.sync.dma_start(out=si2[:], in_=sidx2[:, :])
            dst_sb = cp.tile([P, NT], F32)
            nc.sync.dma_start(out=dst_sb[:], in_=dstc[:, :])
            ad1own = cp.tile([P, NBLK * 8], F16)
            ad2own = cp.tile([P, NBLK], F16)
            as1own = cp.tile([P, NBLK * 8], F16)
            h1own = cp.tile([P, NBLK * HC], F16)
            as2own = cp.tile([P, NBLK], F16)
            h2own = cp.tile([P, NBLK * NC], F16)
            lhsT65 = cp.tile([W2K, P], F16)
            nc.vector.memset(lhsT65[:], 0.0)
            nc.vector.memset(lhsT65[HC:W2K, :], 1.0)
            zcol = cp.tile([P, 1], F32)
            nc.vector.memset(zcol[:], 0.0)
            scol = cp.tile([P, 1], F32)
            nc.vector.memset(scol[:], ESHIFT)

            # ---- node phase: g1 table for every node --------------------
            XC = 4  # blocks per x-load DMA
            for b in range(NODE_BLKS):
                if b % XC == 0:
                    nxc = min(XC, NODE_BLKS - b)
                    x0 = xp.tile([P, XC * P], F16, tag="x0")
                    nc.sync.dma_start(out=x0[:, 0:nxc * P],
                                      in_=xT[0:P, b * P:(b + nxc) * P])
                    x1 = xp.tile([P, XC * P], F16, tag="x1")
                    nc.sync.dma_start(out=x1[:, 0:nxc * P],
                                      in_=xT[P:2 * P, b * P:(b + nxc) * P])
                j = (b % XC) * P
                ps = psA.tile([P, 8 + HC + 8], F32, tag="psA")
                nc.tensor.matmul(out=ps[:], lhsT=x0[:, j:j + P], rhs=w1a0[:], start=True, stop=False)
                nc.tensor.matmul(out=ps[:], lhsT=x1[:, j:j + P], rhs=w1a1[:], start=False, stop=False)
                nc.tensor.matmul(out=ps[:], lhsT=ones_sb[:], rhs=b1a[:], start=False, stop=True)
                st = stp.tile([P, G1W], F16, tag="g1st")
                nc.vector.tensor_copy(out=st[:], in_=ps[:, 0:G1W])
                nc.sync.dma_start(out=g1tab[b * P:(b + 1) * P, :], in_=st[:])
                if b < NBLK:
                    nc.vector.tensor_copy(out=ad1own[:, b * 8:(b + 1) * 8],
                                          in_=ps[:, G1W:G1W + 8])
                    nc.vector.tensor_copy(out=as1own[:, b * 8:(b + 1) * 8],
                                          in_=ps[:, 0:8])
                    nc.vector.tensor_copy(out=h1own[:, b * HC:(b + 1) * HC],
                                          in_=ps[:, 8:8 + HC])

            # ---- layer 1 edge phase + epilogue --------------------------
            t0 = 0
            for b in range(NBLK):
                tb = int(T_B[b])
                g1c = gp.tile([P, tb * G1W], F16, tag="g1c")
                for k in range(tb):
                    nc.gpsimd.indirect_dma_start(
                        out=g1c[:, k * G1W:(k + 1) * G1W], out_offset=None,
                        in_=g1tab[:, :],
                        in_offset=bass.IndirectOffsetOnAxis(
                            ap=si1[:, t0 + k:t0 + k + 1], axis=0))
                ohc = ohp.tile([P, tb * P], F16, tag="ohc")
                nc.vector.tensor_tensor(
                    out=_ap(ohc, 0, [[P, tb], [1, P]]),
                    in0=_ap(iota_sb, 0, [[0, tb], [1, P]]),
                    in1=_ap(dst_sb, t0, [[1, tb], [0, P]]),
                    op=ALU.is_equal)
                dsr = stp.tile([1, tb * P], F16, tag="dsr")
                nc.sync.dma_start(out=dsr[:], in_=dstrow[0:1, t0 * P:(t0 + tb) * P])
                ohtc = ohtp.tile([P, tb * P], F16, tag="ohtc")
                for g in range(0, tb, 4):
                    gw = min(4, tb - g)
                    bps = psW.tile([P, 4 * P], F32, tag="psW")
                    nc.tensor.matmul(out=bps[:, 0:gw * P], lhsT=ones_sb[:],
                                     rhs=dsr[0:1, g * P:(g + gw) * P],
                                     start=True, stop=True)
                    nc.vector.tensor_tensor(
                        out=ohtc[:, g * P:(g + gw) * P], in0=bps[:, 0:gw * P],
                        in1=_ap(iotac_sb, 0, [[0, gw * P]]),
                        op=ALU.is_equal)
                adps = psB.tile([P, tb * 8], F32, tag="psB")
                for k in range(tb):
                    nc.tensor.matmul(out=adps[:, k * 8:(k + 1) * 8],
                                     lhsT=ohtc[:, k * P:(k + 1) * P],
                                     rhs=ad1own[:, b * 8:(b + 1) * 8],
                                     start=True, stop=True)
                ech = ep.tile([P, tb * 8], F32, tag="ech")
                nc.vector.tensor_tensor(
                    out=_ap(ech, 0, [[8, tb], [1, 8]]),
                    in0=_ap(g1c, 0, [[G1W, tb], [1, 8]]),
                    in1=_ap(adps, 0, [[8, tb], [1, 8]]),
                    op=ALU.add)
                lrch = ep.tile([P, tb * 8], F32, tag="lrch")
                nc.vector.scalar_tensor_tensor(out=lrch[:], in0=ech[:], scalar=0.2,
                                               in1=ech[:], op0=ALU.mult, op1=ALU.max)
                pch = ep.tile([P, tb * 8], F32, tag="pch")
                nc.scalar.activation(pch[:], lrch[:], ACTF.Exp, bias=scol[:, 0:1])
                vc = vp.tile([P, tb * G1W], F16, tag="vc")
                nc.vector.tensor_copy(
                    out=_ap(vc, 0, [[G1W, tb], [1, 8]]),
                    in_=_ap(pch, 0, [[8, tb], [1, 8]]))
                nc.vector.tensor_tensor(
                    out=_ap(vc, 8, [[G1W, tb], [8, H], [1, C]]),
                    in0=_ap(g1c, 8, [[G1W, tb], [8, H], [1, C]]),
                    in1=_ap(pch, 0, [[8, tb], [1, H], [0, C]]),
                    op=ALU.mult)
                psagg = psA.tile([P, G1W], F32, tag="psA")
                for k in range(tb):
                    nc.tensor.matmul(out=psagg[:], lhsT=ohc[:, k * P:(k + 1) * P],
                                     rhs=vc[:, k * G1W:(k + 1) * G1W],
                                     start=(k == 0), stop=(k == tb - 1))
                # epilogue: self-loop term, segment-softmax normalize, ELU
                es = ep.tile([P, 8], F32, tag="es")
                nc.vector.tensor_tensor(out=es[:], in0=as1own[:, b * 8:(b + 1) * 8],
                                        in1=ad1own[:, b * 8:(b + 1) * 8], op=ALU.add)
                lrs = ep.tile([P, 8], F32, tag="lrs")
                nc.vector.scalar_tensor_tensor(out=lrs[:], in0=es[:], scalar=0.2,
                                               in1=es[:], op0=ALU.mult, op1=ALU.max)
                psf = ep.tile([P, 8], F32, tag="psf")
                nc.scalar.activation(psf[:], lrs[:], ACTF.Exp, bias=scol[:, 0:1])
                st_ = ep.tile([P, 8], F32, tag="st_")
                nc.vector.tensor_tensor(out=st_[:], in0=psagg[:, 0:8], in1=psf[:], op=ALU.add)
                ssb = ep.tile([P, 8], F32, tag="ssb")
                nc.vector.tensor_scalar_add(out=ssb[:], in0=st_[:], scalar1=1e-16)
                sinv = ep.tile([P, 8], F32, tag="sinv")
                nc.vector.reciprocal(out=sinv[:], in_=ssb[:])
                hw = ep.tile([P, HC], F32, tag="hw")
                nc.vector.tensor_tensor(
                    out=_ap(hw, 0, [[C, H], [1, C]]),
                    in0=_ap(h1own, b * HC, [[C, H], [1, C]]),
                    in1=_ap(psf, 0, [[1, H], [0, C]]),
                    op=ALU.mult)
                wf = ep.tile([P, HC], F32, tag="wf")
                nc.vector.tensor_tensor(out=wf[:], in0=psagg[:, 8:8 + HC], in1=hw[:], op=ALU.add)
                h1f = ep.tile([P, HC], F32, tag="h1f")
                nc.vector.tensor_tensor(
                    out=_ap(h1f, 0, [[C, H], [1, C]]),
                    in0=_ap(wf, 0, [[C, H], [1, C]]),
                    in1=_ap(sinv, 0, [[1, H], [0, C]]),
                    op=ALU.mult)
                t1 = ep.tile([P, HC], F32, tag="t1")
                nc.vector.tensor_scalar_min(out=t1[:], in0=h1f[:], scalar1=0.0)
                t2 = ep.tile([P, HC], F32, tag="t2")
                nc.scalar.activation(t2[:], t1[:], ACTF.Exp, bias=zcol[:, 0:1])
                t3 = ep.tile([P, HC], F32, tag="t3")
                nc.vector.tensor_scalar_max(out=t3[:], in0=h1f[:], scalar1=0.0)
                t4 = ep.tile([P, HC], F32, tag="t4")
                nc.vector.tensor_tensor(out=t4[:], in0=t2[:], in1=t3[:], op=ALU.add)
                h1e = ep.tile([P, HC], F16, tag="h1e")
                nc.vector.tensor_scalar_add(out=h1e[:], in0=t4[:], scalar1=-1.0)
                trp = psT.tile([HC, P], F16, tag="psT")
                nc.tensor.transpose(out=trp[:], in_=h1e[:], identity=ident_sb[:])
                nc.vector.tensor_copy(out=lhsT65[0:HC, :], in_=trp[:])
                ps2 = psB.tile([P, G2W], F32, tag="psB")
                nc.tensor.matmul(out=ps2[:], lhsT=lhsT65[:], rhs=w2a[:],
                                 start=True, stop=True)
                g2st = stp.tile([P, G2W], F16, tag="g2st")
                nc.vector.tensor_copy(out=g2st[:], in_=ps2[:])
                nc.sync.dma_start(out=g2own[b * P:(b + 1) * P, :], in_=g2st[:])
                nc.vector.tensor_copy(out=ad2own[:, b:b + 1], in_=ps2[:, G2R:G2R + 1])
                nc.vector.tensor_copy(out=as2own[:, b:b + 1], in_=ps2[:, 0:1])
                nc.vector.tensor_copy(out=h2own[:, b * NC:(b + 1) * NC], in_=ps2[:, 1:1 + NC])
                t0 += tb

            # ---- halo exchange of layer-2 node table --------------------
            nc.gpsimd.collective_compute(
                "AllGather", ALU.bypass,
                ins=[g2own[:].opt()], outs=[g2full[:].opt()],
                replica_groups=[list(range(CORES))])

            # ---- layer 2 edge phase + epilogue --------------------------
            t0 = 0
            for b in range(NBLK):
                tb = int(T_B[b])
                g2c = gp.tile([P, tb * G2R], F16, tag="g2c")
                for k in range(tb):
                    nc.gpsimd.indirect_dma_start(
                        out=g2c[:, k * G2R:(k + 1) * G2R], out_offset=None,
                        in_=g2full[:, :],
                        in_offset=bass.IndirectOffsetOnAxis(
                            ap=si2[:, t0 + k:t0 + k + 1], axis=0))
                ohc = ohp.tile([P, tb * P], F16, tag="ohc")
                nc.vector.tensor_tensor(
                    out=_ap(ohc, 0, [[P, tb], [1, P]]),
                    in0=_ap(iota_sb, 0, [[0, tb], [1, P]]),
                    in1=_ap(dst_sb, t0, [[1, tb], [0, P]]),
                    op=ALU.is_equal)
                dsr = stp.tile([1, tb * P], F16, tag="dsr")
                nc.sync.dma_start(out=dsr[:], in_=dstrow[0:1, t0 * P:(t0 + tb) * P])
                ohtc = ohtp.tile([P, tb * P], F16, tag="ohtc")
                for g in range(0, tb, 4):
                    gw = min(4, tb - g)
                    bps = psW.tile([P, 4 * P], F32, tag="psW")
                    nc.tensor.matmul(out=bps[:, 0:gw * P], lhsT=ones_sb[:],
                                     rhs=dsr[0:1, g * P:(g + gw) * P],
                                     start=True, stop=True)
                    nc.vector.tensor_tensor(
                        out=ohtc[:, g * P:(g + gw) * P], in0=bps[:, 0:gw * P],
                        in1=_ap(iotac_sb, 0, [[0, gw * P]]),
                        op=ALU.is_equal)
                adps = psB.tile([P, tb], F32, tag="psB")
                for k in range(tb):
                    nc.tensor.matmul(out=adps[:, k:k + 1],
                                     lhsT=ohtc[:, k * P:(k + 1) * P],
                                     rhs=ad2own[:, b:b + 1], start=True, stop=True)
                ech = ep.tile([P, tb], F32, tag="ech")
                nc.vector.tensor_tensor(
                    out=ech[:],
                    in0=_ap(g2c, 0, [[G2R, tb]]),
                    in1=adps[:],
                    op=ALU.add)
                lrch = ep.tile([P, tb], F32, tag="lrch")
                nc.vector.scalar_tensor_tensor(out=lrch[:], in0=ech[:], scalar=0.2,
                                               in1=ech[:], op0=ALU.mult, op1=ALU.max)
                pch = ep.tile([P, tb], F32, tag="pch")
                nc.scalar.activation(pch[:], lrch[:], ACTF.Exp, bias=zcol[:, 0:1])
                vc = vp.tile([P, tb * G2R], F16, tag="vc")
                nc.vector.tensor_copy(out=_ap(vc, 0, [[G2R, tb]]), in_=pch[:])
                for k in range(tb):
                    nc.vector.tensor_scalar_mul(
                        out=vc[:, k * G2R + 1:(k + 1) * G2R],
                        in0=g2c[:, k * G2R + 1:(k + 1) * G2R],
                        scalar1=pch[:, k:k + 1])
                psagg = psA.tile([P, G2R], F32, tag="psA")
                for k in range(tb):
                    nc.tensor.matmul(out=psagg[:], lhsT=ohc[:, k * P:(k + 1) * P],
                                     rhs=vc[:, k * G2R:(k + 1) * G2R],
                                     start=(k == 0), stop=(k == tb - 1))
                es = ep.tile([P, 1], F32, tag="es")
                nc.vector.tensor_tensor(out=es[:], in0=as2own[:, b:b + 1],
                                        in1=ad2own[:, b:b + 1], op=ALU.add)
                lrs = ep.tile([P, 1], F32, tag="lrs")
                nc.vector.scalar_tensor_tensor(out=lrs[:], in0=es[:], scalar=0.2,
                                               in1=es[:], op0=ALU.mult, op1=ALU.max)
                psf = ep.tile([P, 1], F32, tag="psf")
                nc.scalar.activation(psf[:], lrs[:], ACTF.Exp, bias=zcol[:, 0:1])
                st_ = ep.tile([P, 1], F32, tag="st_")
                nc.vector.tensor_tensor(out=st_[:], in0=psagg[:, 0:1], in1=psf[:], op=ALU.add)
                ssb = ep.tile([P, 1], F32, tag="ssb")
                nc.vector.tensor_scalar_add(out=ssb[:], in0=st_[:], scalar1=1e-16)
                sinv = ep.tile([P, 1], F32, tag="sinv")
                nc.vector.reciprocal(out=sinv[:], in_=ssb[:])
                hw2 = ep.tile([P, NC], F32, tag="hw")
                nc.vector.tensor_scalar_mul(out=hw2[:], in0=h2own[:, b * NC:(b + 1) * NC],
                                            scalar1=psf[:, 0:1])
                wf2 = ep.tile([P, NC], F32, tag="wf")
                nc.vector.tensor_tensor(out=wf2[:], in0=psagg[:, 1:1 + NC], in1=hw2[:], op=ALU.add)
                lg = ep.tile([P, NC], F32, tag="t1")
                nc.vector.tensor_scalar_mul(out=lg[:], in0=wf2[:], scalar1=sinv[:, 0:1])
                mx = ep.tile([P, 1], F32, tag="mx")
                nc.vector.reduce_max(mx[:], lg[:], axis=AX.X)
                sh = ep.tile([P, NC], F32, tag="t2")
                nc.vector.tensor_scalar_sub(out=sh[:], in0=lg[:], scalar1=mx[:, 0:1])
                ex = ep.tile([P, NC], F32, tag="t3")
                nc.scalar.activation(ex[:], sh[:], ACTF.Exp, bias=zcol[:, 0:1])
                sm = ep.tile([P, 1], F32, tag="sm")
                nc.vector.reduce_sum(sm[:], ex[:], axis=AX.X)
                ls = ep.tile([P, 1], F32, tag="ls")
                nc.scalar.activation(ls[:], sm[:], ACTF.Ln, bias=zcol[:, 0:1])
                ob = ep.tile([P, NC], F32, tag="t4")
                nc.vector.tensor_scalar_sub(out=ob[:], in0=sh[:], scalar1=ls[:, 0:1])
                nc.sync.dma_start(out=out[b * P:(b + 1) * P, :], in_=ob[:])
                t0 += tb

    nc.compile()
    return nc


def _prep(x, edge_src, edge_dst, W1, a1_src, a1_dst, b1, W2, a2_src, a2_dst, b2):
    """Host-side integer preprocessing (graph partitioning) + param folding."""
    N, F = x.shape
    H, C = a1_src.shape
    NC = W2.shape[1]
    HC = H * C
    NOWN = N // CORES
    NBLK = math.ceil(NOWN / P)
    OWNPAD = NBLK * P
    NFOR = N - NOWN
    FBLK = math.ceil(NFOR / P)
    NODE_BLKS = NBLK + FBLK
    NTAB = NODE_BLKS * P

    # self-loop edges are handled analytically in the block epilogues
    src_all = edge_src
    dst_all = edge_dst

    # per (core, block) edge lists
    core_of = dst_all // NOWN
    per_core = []
    cnt = np.zeros((CORES, NBLK), np.int64)
    for c in range(CORES):
        m = core_of == c
        s, d = src_all[m], dst_all[m] - c * NOWN
        blk = d // P
        order = np.argsort(blk, kind='stable')
        s, d, blk = s[order], d[order], blk[order]
        cnt[c] = np.bincount(blk, minlength=NBLK)
        per_core.append((s, d, blk))
    T_B = np.maximum(1, np.ceil(cnt.max(axis=0) / P).astype(np.int64))
    NT = int(T_B.sum())
    toff = np.concatenate([[0], np.cumsum(T_B)])

    # param folding
    W1r = W1.reshape(F, H, C)
    wsrc = (W1r * a1_src[None]).sum(-1)          # [F, H]
    wdst = (W1r * a1_dst[None]).sum(-1)          # [F, H]
    w1aug = np.concatenate([wsrc, W1, wdst], axis=1).astype(np.float16)   # [F, 8+HC+8]
    b1aug = np.zeros((1, 8 + HC + 8), np.float16)
    b1aug[0, 8:8 + HC] = b1.astype(np.float16)
    G2W = 1 + NC + 1 + 6
    W2K = HC + 1
    w2aug = np.zeros((W2K, G2W), np.float16)
    w2aug[0:HC, 0] = (W2 @ a2_src[0]).astype(np.float16)
    w2aug[0:HC, 1:1 + NC] = W2.astype(np.float16)
    w2aug[0:HC, 1 + NC] = (W2 @ a2_dst[0]).astype(np.float16)
    w2aug[HC, 1:1 + NC] = b2.astype(np.float16)
    ones1 = np.ones((1, P), np.float16)
    iotaf = np.tile(np.arange(P, dtype=np.float32)[None, :], (P, 1))
    ident = np.eye(P, dtype=np.float16)

    xT = np.ascontiguousarray(x.T)               # [F, N] float32

    in_maps = []
    for c in range(CORES):
        own_lo, own_hi = c * NOWN, (c + 1) * NOWN
        # perm: table position -> node
        xTp = np.zeros((F, NTAB), np.float16)
        xTp[:, 0:NOWN] = xT[:, own_lo:own_hi].astype(np.float16)
        fore = np.concatenate([np.arange(0, own_lo), np.arange(own_hi, N)])
        xTp[:, OWNPAD:OWNPAD + NFOR] = xT[:, fore].astype(np.float16)
        # node -> table position
        pos = np.empty(N, np.int64)
        pos[own_lo:own_hi] = np.arange(NOWN)
        pos[fore] = OWNPAD + np.arange(NFOR)

        s, d, blk = per_core[c]
        sidx1 = np.zeros((P, NT), np.int32)
        sidx2 = np.zeros((P, NT), np.int32)
        dstc = np.full((P, NT), -1.0, np.float32)
        bstart = np.concatenate([[0], np.cumsum(np.bincount(blk, minlength=NBLK))])
        for b in range(NBLK):
            eb = slice(bstart[b], bstart[b + 1])
            sb_, db_ = s[eb], d[eb]
            n = len(sb_)
            for k in range(int(T_B[b])):
                lo, hi = k * P, min((k + 1) * P, n)
                if lo >= n:
                    break
                t = toff[b] + k
                m = hi - lo
                sidx1[0:m, t] = pos[sb_[lo:hi]]
                sidx2[0:m, t] = (sb_[lo:hi] // NOWN) * OWNPAD + (sb_[lo:hi] % NOWN)
                dstc[0:m, t] = (db_[lo:hi] % P).astype(np.float32)
        in_maps.append({
            "xT": xTp, "w1aug": w1aug, "b1aug": b1aug, "w2aug": w2aug,
            "ones1": ones1, "iotaf": iotaf.astype(np.float32), "ident": ident,
            "sidx1": sidx1, "sidx2": sidx2, "dstc": dstc,
            "dstrow": dstc.T.reshape(1, NT * P).astype(np.float16),
            "iotac": np.arange(P, dtype=np.float32)[:, None],
        })
    meta = dict(N=N, F=F, H=H, C=C, NC=NC, T_B=T_B, NTAB=NTAB, NBLK=NBLK,
                NODE_BLKS=NODE_BLKS, NOWN=NOWN)
    return in_maps, meta


_CACHED = {}


def run(inputs, eshift=-4.0, trace=False, tmpdir=None):
    in_maps, meta = _prep(**inputs)
    key = (meta["N"], meta["F"], meta["NC"], tuple(meta["T_B"]))
    if key not in _CACHED:
        _CACHED[key] = _build_program(meta["N"], meta["F"], meta["H"], meta["C"],
                                      meta["NC"], meta["T_B"], meta["NTAB"],
                                      meta["NBLK"], meta["NODE_BLKS"], eshift)
    nc = _CACHED[key]
    kw = {"tmpdir": tmpdir} if tmpdir else {}
    res = bass_utils.run_bass_kernel_spmd(nc, in_maps,
                                          core_ids=list(range(CORES)),
                                          trace=trace, **kw)
    outs = [res.results[c]["out"][:meta["NOWN"]] for c in range(CORES)]
    full = np.concatenate(outs, axis=0).astype(np.float32)
    return full, res


def kernel(**inputs):
    full, _ = run(inputs)
    return full


# revision 7
# speedup vs baseline: 1.2086x; 1.0281x over previous
"""2-layer GAT (PyG-style GATConv x2 + log_softmax) on 8 Trainium2 NeuronCores.

Sharding: dst-node sharding (each core owns N/8 destination nodes and all
edges into them). Node features (x) are replicated; each core computes the
full layer-1 node transform, so the only cross-core exchange is one
AllGather of the small layer-2 per-node table between layers.

Edge phase per core: edges sorted by dst block (128 dst nodes per block),
tiles of 128 edges. Per tile: one indirect DMA gathers the [as1|h] rows of
the edge sources from a DRAM table; ad1[dst] is reconstructed on-chip with
a one-hot matmul (no second gather); attention weights p = exp(lrelu(as+ad))
are computed chunked per block; a one-hot aggregation matmul accumulates
[p | p*h] into the per-block PSUM, which is then normalized (segment
softmax) without materializing per-edge alphas.
"""
import sys
sys.path.insert(0, '/opt/trn_rl_repo')
if '/root/.axon_site' not in sys.path:
    sys.path.insert(0, '/root/.axon_site')

import math
import numpy as np

import concourse.bass as bass
import concourse.bacc as bacc
import concourse.tile as tile
from concourse import mybir
from concourse import bass_utils

F16 = mybir.dt.float16
F32 = mybir.dt.float32
I32 = mybir.dt.int32
AX = mybir.AxisListType
ALU = mybir.AluOpType
ACTF = mybir.ActivationFunctionType

CORES = 8
P = 128


def _ap(t, off, dims):
    """AP over pool tile t: partition dim from the tile + given free dims."""
    base = t[:]
    return bass.AP(base.tensor, base.offset + off, [list(base.ap[0])] + [list(d) for d in dims])


def _build_program(N, F, H, C, NC, T_B, NTAB, NBLK, NODE_BLKS, ESHIFT):
    """Build the SPMD Bass program (identical across cores)."""
    HC = H * C
    OWNPAD = NBLK * P
    NT = int(sum(T_B))
    G1W = 8 + HC            # [as1 | h] row width (72)
    G2W = 1 + NC + 1 + 6    # [as2 | h2 | ad2 | pad] = 48
    G2R = 1 + NC            # gathered part of a g2 row (41)
    W2K = HC + 1            # 65

    nc = bacc.Bacc("TRN2", target_bir_lowering=False, debug=False,
                   num_devices=CORES)

    xT = nc.dram_tensor("xT", [F, NODE_BLKS * P], F16, kind="ExternalInput").ap()
    w1aug = nc.dram_tensor("w1aug", [F, 8 + HC + 8], F16, kind="ExternalInput").ap()
    b1aug = nc.dram_tensor("b1aug", [1, 8 + HC + 8], F16, kind="ExternalInput").ap()
    w2aug = nc.dram_tensor("w2aug", [W2K, G2W], F16, kind="ExternalInput").ap()
    ones1 = nc.dram_tensor("ones1", [1, P], F16, kind="ExternalInput").ap()
    iotaf = nc.dram_tensor("iotaf", [P, P], F32, kind="ExternalInput").ap()
    ident = nc.dram_tensor("ident", [P, P], F16, kind="ExternalInput").ap()
    sidx1 = nc.dram_tensor("sidx1", [P, NT], I32, kind="ExternalInput").ap()
    sidx2 = nc.dram_tensor("sidx2", [P, NT], I32, kind="ExternalInput").ap()
    dstc = nc.dram_tensor("dstc", [P, NT], F32, kind="ExternalInput").ap()
    dstrow = nc.dram_tensor("dstrow", [1, NT * P], F16, kind="ExternalInput").ap()
    iotac = nc.dram_tensor("iotac", [P, 1], F32, kind="ExternalInput").ap()
    out = nc.dram_tensor("out", [OWNPAD, NC], F32, kind="ExternalOutput").ap()

    with tile.TileContext(nc) as tc:
        with tc.tile_pool(name="const", bufs=1) as cp, \
             tc.tile_pool(name="xp", bufs=3) as xp, \
             tc.tile_pool(name="stp", bufs=4) as stp, \
             tc.tile_pool(name="gp", bufs=3) as gp, \
             tc.tile_pool(name="ohp", bufs=2) as ohp, \
             tc.tile_pool(name="vp", bufs=2) as vp, \
             tc.tile_pool(name="ohtp", bufs=2) as ohtp, \
             tc.tile_pool(name="ep", bufs=2) as ep, \
             tc.tile_pool(name="psA", bufs=3, space="PSUM") as psA, \
             tc.tile_pool(name="psB", bufs=2, space="PSUM") as psB, \
             tc.tile_pool(name="psT", bufs=1, space="PSUM") as psT, \
             tc.tile_pool(name="psW", bufs=2, space="PSUM") as psW, \
             tc.tile_pool(name="dram", bufs=1, space="DRAM") as dp:

            g1tab = dp.tile([NTAB, G1W], F16)
            g2own = dp.tile([OWNPAD, G2W], F16)
            g2full = dp.tile([CORES * OWNPAD, G2W], F16, addr_space="Shared")

            # ---- resident constants -------------------------------------
            iota_sb = cp.tile([P, P], F32)
            nc.sync.dma_start(out=iota_sb[:], in_=iotaf[:, :])
            iotac_sb = cp.tile([P, 1], F32)
            nc.sync.dma_start(out=iotac_sb[:], in_=iotac[:, :])
            ident_sb = cp.tile([P, P], F16)
            nc.sync.dma_start(out=ident_sb[:], in_=ident[:, :])
            w1a0 = cp.tile([P, 8 + HC + 8], F16)
            nc.sync.dma_start(out=w1a0[:], in_=w1aug[0:P, :])
            w1a1 = cp.tile([P, 8 + HC + 8], F16)
            nc.sync.dma_start(out=w1a1[:], in_=w1aug[P:2 * P, :])
            b1a = cp.tile([1, 8 + HC + 8], F16)
            nc.sync.dma_start(out=b1a[:], in_=b1aug[:, :])
            w2a = cp.tile([W2K, G2W], F16)
            nc.sync.dma_start(out=w2a[:], in_=w2aug[:, :])
            ones_sb = cp.tile([1, P], F16)
            nc.sync.dma_start(out=ones_sb[:], in_=ones1[:, :])
            si1 = cp.tile([P, NT], I32)
            nc.sync.dma_start(out=si1[:], in_=sidx1[:, :])
            si2 = cp.tile([P, NT], I32)
            nc.sync.dma_start(out=si2[:], in_=sidx2[:, :])
            dst_sb = cp.tile([P, NT], F32)
            nc.sync.dma_start(out=dst_sb[:], in_=dstc[:, :])
            ad1own = cp.tile([P, NBLK * 8], F16)
            ad2own = cp.tile([P, NBLK], F16)
            as1own = cp.tile([P, NBLK * 8], F16)
            h1own = cp.tile([P, NBLK * HC], F16)
            as2own = cp.tile([P, NBLK], F16)
            h2own = cp.tile([P, NBLK * NC], F16)
            lhsT65 = cp.tile([W2K, P], F16)
            nc.vector.memset(lhsT65[:], 0.0)
            nc.vector.memset(lhsT65[HC:W2K, :], 1.0)
            zcol = cp.tile([P, 1], F32)
            nc.vector.memset(zcol[:], 0.0)
            scol = cp.tile([P, 1], F32)
            nc.vector.memset(scol[:], ESHIFT)

            # ---- node phase: g1 table for every node --------------------
            XC = 8  # blocks per x-load DMA
            for b in range(NODE_BLKS):
                if b % XC == 0:
                    nxc = min(XC, NODE_BLKS - b)
                    x0 = xp.tile([P, XC * P], F16, tag="x0")
                    nc.sync.dma_start(out=x0[:, 0:nxc * P],
                                      in_=xT[0:P, b * P:(b + nxc) * P])
                    x1 = xp.tile([P, XC * P], F16, tag="x1")
                    nc.sync.dma_start(out=x1[:, 0:nxc * P],
                                      in_=xT[P:2 * P, b * P:(b + nxc) * P])
                j = (b % XC) * P
                ps = psA.tile([P, 8 + HC + 8], F32, tag="psA")
                nc.tensor.matmul(out=ps[:], lhsT=x0[:, j:j + P], rhs=w1a0[:], start=True, stop=False)
                nc.tensor.matmul(out=ps[:], lhsT=x1[:, j:j + P], rhs=w1a1[:], start=False, stop=False)
                nc.tensor.matmul(out=ps[:], lhsT=ones_sb[:], rhs=b1a[:], start=False, stop=True)
                st = stp.tile([P, G1W], F16, tag="g1st")
                nc.vector.tensor_copy(out=st[:], in_=ps[:, 0:G1W])
                nc.sync.dma_start(out=g1tab[b * P:(b + 1) * P, :], in_=st[:])
                if b < NBLK:
                    nc.vector.tensor_copy(out=ad1own[:, b * 8:(b + 1) * 8],
                                          in_=ps[:, G1W:G1W + 8])
                    nc.vector.tensor_copy(out=as1own[:, b * 8:(b + 1) * 8],
                                          in_=ps[:, 0:8])
                    nc.vector.tensor_copy(out=h1own[:, b * HC:(b + 1) * HC],
                                          in_=ps[:, 8:8 + HC])

            # ---- layer 1 edge phase + epilogue --------------------------
            t0 = 0
            for b in range(NBLK):
                tb = int(T_B[b])
                g1c = gp.tile([P, tb * G1W], F16, tag="g1c")
                for k in range(tb):
                    nc.gpsimd.indirect_dma_start(
                        out=g1c[:, k * G1W:(k + 1) * G1W], out_offset=None,
                        in_=g1tab[:, :],
                        in_offset=bass.IndirectOffsetOnAxis(
                            ap=si1[:, t0 + k:t0 + k + 1], axis=0))
                ohc = ohp.tile([P, tb * P], F16, tag="ohc")
                nc.vector.tensor_tensor(
                    out=_ap(ohc, 0, [[P, tb], [1, P]]),
                    in0=_ap(iota_sb, 0, [[0, tb], [1, P]]),
                    in1=_ap(dst_sb, t0, [[1, tb], [0, P]]),
                    op=ALU.is_equal)
                dsr = stp.tile([1, tb * P], F16, tag="dsr")
                nc.sync.dma_start(out=dsr[:], in_=dstrow[0:1, t0 * P:(t0 + tb) * P])
                ohtc = ohtp.tile([P, tb * P], F16, tag="ohtc")
                for g in range(0, tb, 4):
                    gw = min(4, tb - g)
                    bps = psW.tile([P, 4 * P], F32, tag="psW")
                    nc.tensor.matmul(out=bps[:, 0:gw * P], lhsT=ones_sb[:],
                                     rhs=dsr[0:1, g * P:(g + gw) * P],
                                     start=True, stop=True)
                    nc.vector.tensor_tensor(
                        out=ohtc[:, g * P:(g + gw) * P], in0=bps[:, 0:gw * P],
                        in1=_ap(iotac_sb, 0, [[0, gw * P]]),
                        op=ALU.is_equal)
                adps = psB.tile([P, tb * 8], F32, tag="psB")
                for k in range(tb):
                    nc.tensor.matmul(out=adps[:, k * 8:(k + 1) * 8],
                                     lhsT=ohtc[:, k * P:(k + 1) * P],
                                     rhs=ad1own[:, b * 8:(b + 1) * 8],
                                     start=True, stop=True)
                ech = ep.tile([P, tb * 8], F32, tag="ech")
                nc.vector.tensor_tensor(
                    out=_ap(ech, 0, [[8, tb], [1, 8]]),
                    in0=_ap(g1c, 0, [[G1W, tb], [1, 8]]),
                    in1=_ap(adps, 0, [[8, tb], [1, 8]]),
                    op=ALU.add)
                lrch = ep.tile([P, tb * 8], F32, tag="lrch")
                nc.vector.scalar_tensor_tensor(out=lrch[:], in0=ech[:], scalar=0.2,
                                               in1=ech[:], op0=ALU.mult, op1=ALU.max)
                pch = ep.tile([P, tb * 8], F32, tag="pch")
                nc.scalar.activation(pch[:], lrch[:], ACTF.Exp, bias=scol[:, 0:1])
                vc = vp.tile([P, tb * G1W], F16, tag="vc")
                nc.vector.tensor_copy(
                    out=_ap(vc, 0, [[G1W, tb], [1, 8]]),
                    in_=_ap(pch, 0, [[8, tb], [1, 8]]))
                nc.vector.tensor_tensor(
                    out=_ap(vc, 8, [[G1W, tb], [8, H], [1, C]]),
                    in0=_ap(g1c, 8, [[G1W, tb], [8, H], [1, C]]),
                    in1=_ap(pch, 0, [[8, tb], [1, H], [0, C]]),
                    op=ALU.mult)
                psagg = psA.tile([P, G1W], F32, tag="psA")
                for k in range(tb):
                    nc.tensor.matmul(out=psagg[:], lhsT=ohc[:, k * P:(k + 1) * P],
                                     rhs=vc[:, k * G1W:(k + 1) * G1W],
                                     start=(k == 0), stop=(k == tb - 1))
                # epilogue: self-loop term, segment-softmax normalize, ELU
                es = ep.tile([P, 8], F32, tag="es")
                nc.vector.tensor_tensor(out=es[:], in0=as1own[:, b * 8:(b + 1) * 8],
                                        in1=ad1own[:, b * 8:(b + 1) * 8], op=ALU.add)
                lrs = ep.tile([P, 8], F32, tag="lrs")
                nc.vector.scalar_tensor_tensor(out=lrs[:], in0=es[:], scalar=0.2,
                                               in1=es[:], op0=ALU.mult, op1=ALU.max)
                psf = ep.tile([P, 8], F32, tag="psf")
                nc.scalar.activation(psf[:], lrs[:], ACTF.Exp, bias=scol[:, 0:1])
                st_ = ep.tile([P, 8], F32, tag="st_")
                nc.vector.tensor_tensor(out=st_[:], in0=psagg[:, 0:8], in1=psf[:], op=ALU.add)
                ssb = ep.tile([P, 8], F32, tag="ssb")
                nc.vector.tensor_scalar_add(out=ssb[:], in0=st_[:], scalar1=1e-16)
                sinv = ep.tile([P, 8], F32, tag="sinv")
                nc.vector.reciprocal(out=sinv[:], in_=ssb[:])
                hw = ep.tile([P, HC], F32, tag="hw")
                nc.vector.tensor_tensor(
                    out=_ap(hw, 0, [[C, H], [1, C]]),
                    in0=_ap(h1own, b * HC, [[C, H], [1, C]]),
                    in1=_ap(psf, 0, [[1, H], [0, C]]),
                    op=ALU.mult)
                wf = ep.tile([P, HC], F32, tag="wf")
                nc.vector.tensor_tensor(out=wf[:], in0=psagg[:, 8:8 + HC], in1=hw[:], op=ALU.add)
                h1f = ep.tile([P, HC], F32, tag="h1f")
                nc.vector.tensor_tensor(
                    out=_ap(h1f, 0, [[C, H], [1, C]]),
                    in0=_ap(wf, 0, [[C, H], [1, C]]),
                    in1=_ap(sinv, 0, [[1, H], [0, C]]),
                    op=ALU.mult)
                t1 = ep.tile([P, HC], F32, tag="t1")
                nc.vector.tensor_scalar_min(out=t1[:], in0=h1f[:], scalar1=0.0)
                t2 = ep.tile([P, HC], F32, tag="t2")
                nc.scalar.activation(t2[:], t1[:], ACTF.Exp, bias=zcol[:, 0:1])
                t3 = ep.tile([P, HC], F32, tag="t3")
                nc.vector.tensor_scalar_max(out=t3[:], in0=h1f[:], scalar1=0.0)
                t4 = ep.tile([P, HC], F32, tag="t4")
                nc.vector.tensor_tensor(out=t4[:], in0=t2[:], in1=t3[:], op=ALU.add)
                h1e = ep.tile([P, HC], F16, tag="h1e")
                nc.vector.tensor_scalar_add(out=h1e[:], in0=t4[:], scalar1=-1.0)
                trp = psT.tile([HC, P], F16, tag="psT")
                nc.tensor.transpose(out=trp[:], in_=h1e[:], identity=ident_sb[:])
                nc.vector.tensor_copy(out=lhsT65[0:HC, :], in_=trp[:])
                ps2 = psB.tile([P, G2W], F32, tag="psB")
                nc.tensor.matmul(out=ps2[:], lhsT=lhsT65[:], rhs=w2a[:],
                                 start=True, stop=True)
                g2st = stp.tile([P, G2W], F16, tag="g2st")
                nc.vector.tensor_copy(out=g2st[:], in_=ps2[:])
                nc.sync.dma_start(out=g2own[b * P:(b + 1) * P, :], in_=g2st[:])
                nc.vector.tensor_copy(out=ad2own[:, b:b + 1], in_=ps2[:, G2R:G2R + 1])
                nc.vector.tensor_copy(out=as2own[:, b:b + 1], in_=ps2[:, 0:1])
                nc.vector.tensor_copy(out=h2own[:, b * NC:(b + 1) * NC], in_=ps2[:, 1:1 + NC])
                t0 += tb

            # ---- halo exchange of layer-2 node table --------------------
            nc.gpsimd.collective_compute(
                "AllGather", ALU.bypass,
                ins=[g2own[:].opt()], outs=[g2full[:].opt()],
                replica_groups=[list(range(CORES))])

            # ---- layer 2 edge phase + epilogue --------------------------
            t0 = 0
            for b in range(NBLK):
                tb = int(T_B[b])
                g2c = gp.tile([P, tb * G2R], F16, tag="g2c")
                for k in range(tb):
                    nc.gpsimd.indirect_dma_start(
                        out=g2c[:, k * G2R:(k + 1) * G2R], out_offset=None,
                        in_=g2full[:, :],
                        in_offset=bass.IndirectOffsetOnAxis(
                            ap=si2[:, t0 + k:t0 + k + 1], axis=0))
                ohc = ohp.tile([P, tb * P], F16, tag="ohc")
                nc.vector.tensor_tensor(
                    out=_ap(ohc, 0, [[P, tb], [1, P]]),
                    in0=_ap(iota_sb, 0, [[0, tb], [1, P]]),
                    in1=_ap(dst_sb, t0, [[1, tb], [0, P]]),
                    op=ALU.is_equal)
                dsr = stp.tile([1, tb * P], F16, tag="dsr")
                nc.sync.dma_start(out=dsr[:], in_=dstrow[0:1, t0 * P:(t0 + tb) * P])
                ohtc = ohtp.tile([P, tb * P], F16, tag="ohtc")
                for g in range(0, tb, 4):
                    gw = min(4, tb - g)
                    bps = psW.tile([P, 4 * P], F32, tag="psW")
                    nc.tensor.matmul(out=bps[:, 0:gw * P], lhsT=ones_sb[:],
                                     rhs=dsr[0:1, g * P:(g + gw) * P],
                                     start=True, stop=True)
                    nc.vector.tensor_tensor(
                        out=ohtc[:, g * P:(g + gw) * P], in0=bps[:, 0:gw * P],
                        in1=_ap(iotac_sb, 0, [[0, gw * P]]),
                        op=ALU.is_equal)
                adps = psB.tile([P, tb], F32, tag="psB")
                for k in range(tb):
                    nc.tensor.matmul(out=adps[:, k:k + 1],
                                     lhsT=ohtc[:, k * P:(k + 1) * P],
                                     rhs=ad2own[:, b:b + 1], start=True, stop=True)
                ech = ep.tile([P, tb], F32, tag="ech")
                nc.vector.tensor_tensor(
                    out=ech[:],
                    in0=_ap(g2c, 0, [[G2R, tb]]),
                    in1=adps[:],
                    op=ALU.add)
                lrch = ep.tile([P, tb], F32, tag="lrch")
                nc.vector.scalar_tensor_tensor(out=lrch[:], in0=ech[:], scalar=0.2,
                                               in1=ech[:], op0=ALU.mult, op1=ALU.max)
                pch = ep.tile([P, tb], F32, tag="pch")
                nc.scalar.activation(pch[:], lrch[:], ACTF.Exp, bias=zcol[:, 0:1])
                vc = vp.tile([P, tb * G2R], F16, tag="vc")
                nc.vector.tensor_copy(out=_ap(vc, 0, [[G2R, tb]]), in_=pch[:])
                for k in range(tb):
                    nc.vector.tensor_scalar_mul(
                        out=vc[:, k * G2R + 1:(k + 1) * G2R],
                        in0=g2c[:, k * G2R + 1:(k + 1) * G2R],
                        scalar1=pch[:, k:k + 1])
                psagg = psA.tile([P, G2R], F32, tag="psA")
                for k in range(tb):
                    nc.tensor.matmul(out=psagg[:], lhsT=ohc[:, k * P:(k + 1) * P],
                                     rhs=vc[:, k * G2R:(k + 1) * G2R],
                                     start=(k == 0), stop=(k == tb - 1))
                es = ep.tile([P, 1], F32, tag="es")
                nc.vector.tensor_tensor(out=es[:], in0=as2own[:, b:b + 1],
                                        in1=ad2own[:, b:b + 1], op=ALU.add)
                lrs = ep.tile([P, 1], F32, tag="lrs")
                nc.vector.scalar_tensor_tensor(out=lrs[:], in0=es[:], scalar=0.2,
                                               in1=es[:], op0=ALU.mult, op1=ALU.max)
                psf = ep.tile([P, 1], F32, tag="psf")
                nc.scalar.activation(psf[:], lrs[:], ACTF.Exp, bias=zcol[:, 0:1])
                st_ = ep.tile([P, 1], F32, tag="st_")
                nc.vector.tensor_tensor(out=st_[:], in0=psagg[:, 0:1], in1=psf[:], op=ALU.add)
                ssb = ep.tile([P, 1], F32, tag="ssb")
                nc.vector.tensor_scalar_add(out=ssb[:], in0=st_[:], scalar1=1e-16)
                sinv = ep.tile([P, 1], F32, tag="sinv")
                nc.vector.reciprocal(out=sinv[:], in_=ssb[:])
                hw2 = ep.tile([P, NC], F32, tag="hw")
                nc.vector.tensor_scalar_mul(out=hw2[:], in0=h2own[:, b * NC:(b + 1) * NC],
                                            scalar1=psf[:, 0:1])
                wf2 = ep.tile([P, NC], F32, tag="wf")
                nc.vector.tensor_tensor(out=wf2[:], in0=psagg[:, 1:1 + NC], in1=hw2[:], op=ALU.add)
                lg = ep.tile([P, NC], F32, tag="t1")
                nc.vector.tensor_scalar_mul(out=lg[:], in0=wf2[:], scalar1=sinv[:, 0:1])
                mx = ep.tile([P, 1], F32, tag="mx")
                nc.vector.reduce_max(mx[:], lg[:], axis=AX.X)
                sh = ep.tile([P, NC], F32, tag="t2")
                nc.vector.tensor_scalar_sub(out=sh[:], in0=lg[:], scalar1=mx[:, 0:1])
                ex = ep.tile([P, NC], F32, tag="t3")
                nc.scalar.activation(ex[:], sh[:], ACTF.Exp, bias=zcol[:, 0:1])
                sm = ep.tile([P, 1], F32, tag="sm")
                nc.vector.reduce_sum(sm[:], ex[:], axis=AX.X)
                ls = ep.tile([P, 1], F32, tag="ls")
                nc.scalar.activation(ls[:], sm[:], ACTF.Ln, bias=zcol[:, 0:1])
                ob = ep.tile([P, NC], F32, tag="t4")
                nc.vector.tensor_scalar_sub(out=ob[:], in0=sh[:], scalar1=ls[:, 0:1])
                nc.sync.dma_start(out=out[b * P:(b + 1) * P, :], in_=ob[:])
                t0 += tb

    nc.compile()
    return nc


def _prep(x, edge_src, edge_dst, W1, a1_src, a1_dst, b1, W2, a2_src, a2_dst, b2):
    """Host-side integer preprocessing (graph partitioning) + param folding."""
    N, F = x.shape
    H, C = a1_src.shape
    NC = W2.shape[1]
    HC = H * C
    NOWN = N // CORES
    NBLK = math.ceil(NOWN / P)
    OWNPAD = NBLK * P
    NFOR = N - NOWN
    FBLK = math.ceil(NFOR / P)
    NODE_BLKS = NBLK + FBLK
    NTAB = NODE_BLKS * P

    # self-loop edges are handled analytically in the block epilogues
    src_all = edge_src
    dst_all = edge_dst

    # per (core, block) edge lists
    core_of = dst_all // NOWN
    per_core = []
    cnt = np.zeros((CORES, NBLK), np.int64)
    for c in range(CORES):
        m = core_of == c
        s, d = src_all[m], dst_all[m] - c * NOWN
        blk = d // P
        order = np.argsort(blk, kind='stable')
        s, d, blk = s[order], d[order], blk[order]
        cnt[c] = np.bincount(blk, minlength=NBLK)
        per_core.append((s, d, blk))
    T_B = np.maximum(1, np.ceil(cnt.max(axis=0) / P).astype(np.int64))
    NT = int(T_B.sum())
    toff = np.concatenate([[0], np.cumsum(T_B)])

    # param folding
    W1r = W1.reshape(F, H, C)
    wsrc = (W1r * a1_src[None]).sum(-1)          # [F, H]
    wdst = (W1r * a1_dst[None]).sum(-1)          # [F, H]
    w1aug = np.concatenate([wsrc, W1, wdst], axis=1).astype(np.float16)   # [F, 8+HC+8]
    b1aug = np.zeros((1, 8 + HC + 8), np.float16)
    b1aug[0, 8:8 + HC] = b1.astype(np.float16)
    G2W = 1 + NC + 1 + 6
    W2K = HC + 1
    w2aug = np.zeros((W2K, G2W), np.float16)
    w2aug[0:HC, 0] = (W2 @ a2_src[0]).astype(np.float16)
    w2aug[0:HC, 1:1 + NC] = W2.astype(np.float16)
    w2aug[0:HC, 1 + NC] = (W2 @ a2_dst[0]).astype(np.float16)
    w2aug[HC, 1:1 + NC] = b2.astype(np.float16)
    ones1 = np.ones((1, P), np.float16)
    iotaf = np.tile(np.arange(P, dtype=np.float32)[None, :], (P, 1))
    ident = np.eye(P, dtype=np.float16)

    xT = np.ascontiguousarray(x.T)               # [F, N] float32

    in_maps = []
    for c in range(CORES):
        own_lo, own_hi = c * NOWN, (c + 1) * NOWN
        # perm: table position -> node
        xTp = np.zeros((F, NTAB), np.float16)
        xTp[:, 0:NOWN] = xT[:, own_lo:own_hi].astype(np.float16)
        fore = np.concatenate([np.arange(0, own_lo), np.arange(own_hi, N)])
        xTp[:, OWNPAD:OWNPAD + NFOR] = xT[:, fore].astype(np.float16)
        # node -> table position
        pos = np.empty(N, np.int64)
        pos[own_lo:own_hi] = np.arange(NOWN)
        pos[fore] = OWNPAD + np.arange(NFOR)

        s, d, blk = per_core[c]
        sidx1 = np.zeros((P, NT), np.int32)
        sidx2 = np.zeros((P, NT), np.int32)
        dstc = np.full((P, NT), -1.0, np.float32)
        bstart = np.concatenate([[0], np.cumsum(np.bincount(blk, minlength=NBLK))])
        for b in range(NBLK):
            eb = slice(bstart[b], bstart[b + 1])
            sb_, db_ = s[eb], d[eb]
            n = len(sb_)
            for k in range(int(T_B[b])):
                lo, hi = k * P, min((k + 1) * P, n)
                if lo >= n:
                    break
                t = toff[b] + k
                m = hi - lo
                sidx1[0:m, t] = pos[sb_[lo:hi]]
                sidx2[0:m, t] = (sb_[lo:hi] // NOWN) * OWNPAD + (sb_[lo:hi] % NOWN)
                dstc[0:m, t] = (db_[lo:hi] % P).astype(np.float32)
        in_maps.append({
            "xT": xTp, "w1aug": w1aug, "b1aug": b1aug, "w2aug": w2aug,
            "ones1": ones1, "iotaf": iotaf.astype(np.float32), "ident": ident,
            "sidx1": sidx1, "sidx2": sidx2, "dstc": dstc,
            "dstrow": dstc.T.reshape(1, NT * P).astype(np.float16),
            "iotac": np.arange(P, dtype=np.float32)[:, None],
        })
    meta = dict(N=N, F=F, H=H, C=C, NC=NC, T_B=T_B, NTAB=NTAB, NBLK=NBLK,
                NODE_BLKS=NODE_BLKS, NOWN=NOWN)
    return in_maps, meta


_CACHED = {}


def run(inputs, eshift=-4.0, trace=False, tmpdir=None):
    in_maps, meta = _prep(**inputs)
    key = (meta["N"], meta["F"], meta["NC"], tuple(meta["T_B"]))
    if key not in _CACHED:
        _CACHED[key] = _build_program(meta["N"], meta["F"], meta["H"], meta["C"],
                                      meta["NC"], meta["T_B"], meta["NTAB"],
                                      meta["NBLK"], meta["NODE_BLKS"], eshift)
    nc = _CACHED[key]
    kw = {"tmpdir": tmpdir} if tmpdir else {}
    res = bass_utils.run_bass_kernel_spmd(nc, in_maps,
                                          core_ids=list(range(CORES)),
                                          trace=trace, **kw)
    outs = [res.results[c]["out"][:meta["NOWN"]] for c in range(CORES)]
    full = np.concatenate(outs, axis=0).astype(np.float32)
    return full, res


def kernel(**inputs):
    full, _ = run(inputs)
    return full


# revision 8
# speedup vs baseline: 1.2087x; 1.0001x over previous
"""2-layer GAT (PyG-style GATConv x2 + log_softmax) on 8 Trainium2 NeuronCores.

Sharding: dst-node sharding (each core owns N/8 destination nodes and all
edges into them). Node features (x) are replicated; each core computes the
full layer-1 node transform, so the only cross-core exchange is one
AllGather of the small layer-2 per-node table between layers.

Edge phase per core: edges sorted by dst block (128 dst nodes per block),
tiles of 128 edges. Per tile: one indirect DMA gathers the [as1|h] rows of
the edge sources from a DRAM table; ad1[dst] is reconstructed on-chip with
a one-hot matmul (no second gather); attention weights p = exp(lrelu(as+ad))
are computed chunked per block; a one-hot aggregation matmul accumulates
[p | p*h] into the per-block PSUM, which is then normalized (segment
softmax) without materializing per-edge alphas.
"""
import sys
sys.path.insert(0, '/opt/trn_rl_repo')
if '/root/.axon_site' not in sys.path:
    sys.path.insert(0, '/root/.axon_site')

import math
import numpy as np

import concourse.bass as bass
import concourse.bacc as bacc
import concourse.tile as tile
from concourse import mybir
from concourse import bass_utils

F16 = mybir.dt.float16
F32 = mybir.dt.float32
I32 = mybir.dt.int32
AX = mybir.AxisListType
ALU = mybir.AluOpType
ACTF = mybir.ActivationFunctionType

CORES = 8
P = 128


def _ap(t, off, dims):
    """AP over pool tile t: partition dim from the tile + given free dims."""
    base = t[:]
    return bass.AP(base.tensor, base.offset + off, [list(base.ap[0])] + [list(d) for d in dims])


def _build_program(N, F, H, C, NC, T_B, NTAB, NBLK, NODE_BLKS, ESHIFT, HAS_B1):
    """Build the SPMD Bass program (identical across cores)."""
    HC = H * C
    OWNPAD = NBLK * P
    NT = int(sum(T_B))
    G1W = 8 + HC            # [as1 | h] row width (72)
    G2W = 1 + NC + 1 + 6    # [as2 | h2 | ad2 | pad] = 48
    G2R = 1 + NC            # gathered part of a g2 row (41)
    W2K = HC + 1            # 65

    nc = bacc.Bacc("TRN2", target_bir_lowering=False, debug=False,
                   num_devices=CORES)

    xT = nc.dram_tensor("xT", [F, NODE_BLKS * P], F16, kind="ExternalInput").ap()
    w1aug = nc.dram_tensor("w1aug", [F, 8 + HC + 8], F16, kind="ExternalInput").ap()
    b1aug = nc.dram_tensor("b1aug", [1, 8 + HC + 8], F16, kind="ExternalInput").ap()
    w2aug = nc.dram_tensor("w2aug", [W2K, G2W], F16, kind="ExternalInput").ap()
    ones1 = nc.dram_tensor("ones1", [1, P], F16, kind="ExternalInput").ap()
    iotaf = nc.dram_tensor("iotaf", [P, P], F32, kind="ExternalInput").ap()
    ident = nc.dram_tensor("ident", [P, P], F16, kind="ExternalInput").ap()
    sidx1 = nc.dram_tensor("sidx1", [P, NT], I32, kind="ExternalInput").ap()
    sidx2 = nc.dram_tensor("sidx2", [P, NT], I32, kind="ExternalInput").ap()
    dstc = nc.dram_tensor("dstc", [P, NT], F32, kind="ExternalInput").ap()
    dstrow = nc.dram_tensor("dstrow", [1, NT * P], F16, kind="ExternalInput").ap()
    iotac = nc.dram_tensor("iotac", [P, 1], F32, kind="ExternalInput").ap()
    out = nc.dram_tensor("out", [OWNPAD, NC], F32, kind="ExternalOutput").ap()

    with tile.TileContext(nc) as tc:
        with tc.tile_pool(name="const", bufs=1) as cp, \
             tc.tile_pool(name="xp", bufs=3) as xp, \
             tc.tile_pool(name="stp", bufs=4) as stp, \
             tc.tile_pool(name="gp", bufs=3) as gp, \
             tc.tile_pool(name="ohp", bufs=2) as ohp, \
             tc.tile_pool(name="vp", bufs=2) as vp, \
             tc.tile_pool(name="ohtp", bufs=2) as ohtp, \
             tc.tile_pool(name="ep", bufs=2) as ep, \
             tc.tile_pool(name="psA", bufs=3, space="PSUM") as psA, \
             tc.tile_pool(name="psB", bufs=2, space="PSUM") as psB, \
             tc.tile_pool(name="psT", bufs=1, space="PSUM") as psT, \
             tc.tile_pool(name="psW", bufs=2, space="PSUM") as psW, \
             tc.tile_pool(name="dram", bufs=1, space="DRAM") as dp:

            g1tab = dp.tile([NTAB, G1W], F16)
            g2own = dp.tile([OWNPAD, G2W], F16)
            g2full = dp.tile([CORES * OWNPAD, G2W], F16, addr_space="Shared")

            # ---- resident constants -------------------------------------
            iota_sb = cp.tile([P, P], F32)
            nc.sync.dma_start(out=iota_sb[:], in_=iotaf[:, :])
            iotac_sb = cp.tile([P, 1], F32)
            nc.sync.dma_start(out=iotac_sb[:], in_=iotac[:, :])
            ident_sb = cp.tile([P, P], F16)
            nc.sync.dma_start(out=ident_sb[:], in_=ident[:, :])
            w1a0 = cp.tile([P, 8 + HC + 8], F16)
            nc.sync.dma_start(out=w1a0[:], in_=w1aug[0:P, :])
            w1a1 = cp.tile([P, 8 + HC + 8], F16)
            nc.sync.dma_start(out=w1a1[:], in_=w1aug[P:2 * P, :])
            b1a = cp.tile([1, 8 + HC + 8], F16)
            nc.sync.dma_start(out=b1a[:], in_=b1aug[:, :])
            w2a = cp.tile([W2K, G2W], F16)
            nc.sync.dma_start(out=w2a[:], in_=w2aug[:, :])
            ones_sb = cp.tile([1, P], F16)
            nc.sync.dma_start(out=ones_sb[:], in_=ones1[:, :])
            si1 = cp.tile([P, NT], I32)
            nc.sync.dma_start(out=si1[:], in_=sidx1[:, :])
            si2 = cp.tile([P, NT], I32)
            nc.sync.dma_start(out=si2[:], in_=sidx2[:, :])
            dst_sb = cp.tile([P, NT], F32)
            nc.sync.dma_start(out=dst_sb[:], in_=dstc[:, :])
            ad1own = cp.tile([P, NBLK * 8], F16)
            ad2own = cp.tile([P, NBLK], F16)
            as1own = cp.tile([P, NBLK * 8], F16)
            h1own = cp.tile([P, NBLK * HC], F16)
            as2own = cp.tile([P, NBLK], F16)
            h2own = cp.tile([P, NBLK * NC], F16)
            lhsT65 = cp.tile([W2K, P], F16)
            nc.vector.memset(lhsT65[:], 0.0)
            nc.vector.memset(lhsT65[HC:W2K, :], 1.0)
            zcol = cp.tile([P, 1], F32)
            nc.vector.memset(zcol[:], 0.0)
            scol = cp.tile([P, 1], F32)
            nc.vector.memset(scol[:], ESHIFT)

            # ---- node phase: g1 table for every node --------------------
            XC = 8  # blocks per x-load DMA
            for b in range(NODE_BLKS):
                if b % XC == 0:
                    nxc = min(XC, NODE_BLKS - b)
                    x0 = xp.tile([P, XC * P], F16, tag="x0")
                    nc.sync.dma_start(out=x0[:, 0:nxc * P],
                                      in_=xT[0:P, b * P:(b + nxc) * P])
                    x1 = xp.tile([P, XC * P], F16, tag="x1")
                    nc.sync.dma_start(out=x1[:, 0:nxc * P],
                                      in_=xT[P:2 * P, b * P:(b + nxc) * P])
                j = (b % XC) * P
                ps = psA.tile([P, 8 + HC + 8], F32, tag="psA")
                nc.tensor.matmul(out=ps[:], lhsT=x0[:, j:j + P], rhs=w1a0[:], start=True, stop=False)
                nc.tensor.matmul(out=ps[:], lhsT=x1[:, j:j + P], rhs=w1a1[:], start=False,
                                 stop=not HAS_B1)
                if HAS_B1:
                    nc.tensor.matmul(out=ps[:], lhsT=ones_sb[:], rhs=b1a[:], start=False, stop=True)
                st = stp.tile([P, G1W], F16, tag="g1st")
                nc.vector.tensor_copy(out=st[:], in_=ps[:, 0:G1W])
                nc.sync.dma_start(out=g1tab[b * P:(b + 1) * P, :], in_=st[:])
                if b < NBLK:
                    nc.vector.tensor_copy(out=ad1own[:, b * 8:(b + 1) * 8],
                                          in_=ps[:, G1W:G1W + 8])
                    nc.vector.tensor_copy(out=as1own[:, b * 8:(b + 1) * 8],
                                          in_=ps[:, 0:8])
                    nc.vector.tensor_copy(out=h1own[:, b * HC:(b + 1) * HC],
                                          in_=ps[:, 8:8 + HC])

            # ---- layer 1 edge phase + epilogue --------------------------
            t0 = 0
            for b in range(NBLK):
                tb = int(T_B[b])
                g1c = gp.tile([P, tb * G1W], F16, tag="g1c")
                for k in range(tb):
                    nc.gpsimd.indirect_dma_start(
                        out=g1c[:, k * G1W:(k + 1) * G1W], out_offset=None,
                        in_=g1tab[:, :],
                        in_offset=bass.IndirectOffsetOnAxis(
                            ap=si1[:, t0 + k:t0 + k + 1], axis=0))
                ohc = ohp.tile([P, tb * P], F16, tag="ohc")
                nc.vector.tensor_tensor(
                    out=_ap(ohc, 0, [[P, tb], [1, P]]),
                    in0=_ap(iota_sb, 0, [[0, tb], [1, P]]),
                    in1=_ap(dst_sb, t0, [[1, tb], [0, P]]),
                    op=ALU.is_equal)
                dsr = stp.tile([1, tb * P], F16, tag="dsr")
                nc.sync.dma_start(out=dsr[:], in_=dstrow[0:1, t0 * P:(t0 + tb) * P])
                ohtc = ohtp.tile([P, tb * P], F16, tag="ohtc")
                for g in range(0, tb, 4):
                    gw = min(4, tb - g)
                    bps = psW.tile([P, 4 * P], F32, tag="psW")
                    nc.tensor.matmul(out=bps[:, 0:gw * P], lhsT=ones_sb[:],
                                     rhs=dsr[0:1, g * P:(g + gw) * P],
                                     start=True, stop=True)
                    nc.vector.tensor_tensor(
                        out=ohtc[:, g * P:(g + gw) * P], in0=bps[:, 0:gw * P],
                        in1=_ap(iotac_sb, 0, [[0, gw * P]]),
                        op=ALU.is_equal)
                adps = psB.tile([P, tb * 8], F32, tag="psB")
                for k in range(tb):
                    nc.tensor.matmul(out=adps[:, k * 8:(k + 1) * 8],
                                     lhsT=ohtc[:, k * P:(k + 1) * P],
                                     rhs=ad1own[:, b * 8:(b + 1) * 8],
                                     start=True, stop=True)
                ech = ep.tile([P, tb * 8], F32, tag="ech")
                nc.vector.tensor_tensor(
                    out=_ap(ech, 0, [[8, tb], [1, 8]]),
                    in0=_ap(g1c, 0, [[G1W, tb], [1, 8]]),
                    in1=_ap(adps, 0, [[8, tb], [1, 8]]),
                    op=ALU.add)
                lrch = ep.tile([P, tb * 8], F32, tag="lrch")
                nc.vector.scalar_tensor_tensor(out=lrch[:], in0=ech[:], scalar=0.2,
                                               in1=ech[:], op0=ALU.mult, op1=ALU.max)
                pch = ep.tile([P, tb * 8], F32, tag="pch")
                nc.scalar.activation(pch[:], lrch[:], ACTF.Exp, bias=scol[:, 0:1])
                vc = vp.tile([P, tb * G1W], F16, tag="vc")
                nc.vector.tensor_copy(
                    out=_ap(vc, 0, [[G1W, tb], [1, 8]]),
                    in_=_ap(pch, 0, [[8, tb], [1, 8]]))
                nc.vector.tensor_tensor(
                    out=_ap(vc, 8, [[G1W, tb], [8, H], [1, C]]),
                    in0=_ap(g1c, 8, [[G1W, tb], [8, H], [1, C]]),
                    in1=_ap(pch, 0, [[8, tb], [1, H], [0, C]]),
                    op=ALU.mult)
                psagg = psA.tile([P, G1W], F32, tag="psA")
                for k in range(tb):
                    nc.tensor.matmul(out=psagg[:], lhsT=ohc[:, k * P:(k + 1) * P],
                                     rhs=vc[:, k * G1W:(k + 1) * G1W],
                                     start=(k == 0), stop=(k == tb - 1))
                # epilogue: self-loop term, segment-softmax normalize, ELU
                es = ep.tile([P, 8], F32, tag="es")
                nc.vector.tensor_tensor(out=es[:], in0=as1own[:, b * 8:(b + 1) * 8],
                                        in1=ad1own[:, b * 8:(b + 1) * 8], op=ALU.add)
                lrs = ep.tile([P, 8], F32, tag="lrs")
                nc.vector.scalar_tensor_tensor(out=lrs[:], in0=es[:], scalar=0.2,
                                               in1=es[:], op0=ALU.mult, op1=ALU.max)
                psf = ep.tile([P, 8], F32, tag="psf")
                nc.scalar.activation(psf[:], lrs[:], ACTF.Exp, bias=scol[:, 0:1])
                st_ = ep.tile([P, 8], F32, tag="st_")
                nc.vector.tensor_tensor(out=st_[:], in0=psagg[:, 0:8], in1=psf[:], op=ALU.add)
                ssb = ep.tile([P, 8], F32, tag="ssb")
                nc.vector.tensor_scalar_add(out=ssb[:], in0=st_[:], scalar1=1e-16)
                sinv = ep.tile([P, 8], F32, tag="sinv")
                nc.vector.reciprocal(out=sinv[:], in_=ssb[:])
                hw = ep.tile([P, HC], F32, tag="hw")
                nc.vector.tensor_tensor(
                    out=_ap(hw, 0, [[C, H], [1, C]]),
                    in0=_ap(h1own, b * HC, [[C, H], [1, C]]),
                    in1=_ap(psf, 0, [[1, H], [0, C]]),
                    op=ALU.mult)
                wf = ep.tile([P, HC], F32, tag="wf")
                nc.vector.tensor_tensor(out=wf[:], in0=psagg[:, 8:8 + HC], in1=hw[:], op=ALU.add)
                h1f = ep.tile([P, HC], F32, tag="h1f")
                nc.vector.tensor_tensor(
                    out=_ap(h1f, 0, [[C, H], [1, C]]),
                    in0=_ap(wf, 0, [[C, H], [1, C]]),
                    in1=_ap(sinv, 0, [[1, H], [0, C]]),
                    op=ALU.mult)
                t1 = ep.tile([P, HC], F32, tag="t1")
                nc.vector.tensor_scalar_min(out=t1[:], in0=h1f[:], scalar1=0.0)
                t2 = ep.tile([P, HC], F32, tag="t2")
                nc.scalar.activation(t2[:], t1[:], ACTF.Exp, bias=zcol[:, 0:1])
                t3 = ep.tile([P, HC], F32, tag="t3")
                nc.vector.tensor_scalar_max(out=t3[:], in0=h1f[:], scalar1=0.0)
                t4 = ep.tile([P, HC], F32, tag="t4")
                nc.vector.tensor_tensor(out=t4[:], in0=t2[:], in1=t3[:], op=ALU.add)
                h1e = ep.tile([P, HC], F16, tag="h1e")
                nc.vector.tensor_scalar_add(out=h1e[:], in0=t4[:], scalar1=-1.0)
                trp = psT.tile([HC, P], F16, tag="psT")
                nc.tensor.transpose(out=trp[:], in_=h1e[:], identity=ident_sb[:])
                nc.vector.tensor_copy(out=lhsT65[0:HC, :], in_=trp[:])
                ps2 = psB.tile([P, G2W], F32, tag="psB")
                nc.tensor.matmul(out=ps2[:], lhsT=lhsT65[:], rhs=w2a[:],
                                 start=True, stop=True)
                g2st = stp.tile([P, G2W], F16, tag="g2st")
                nc.vector.tensor_copy(out=g2st[:], in_=ps2[:])
                nc.sync.dma_start(out=g2own[b * P:(b + 1) * P, :], in_=g2st[:])
                nc.vector.tensor_copy(out=ad2own[:, b:b + 1], in_=ps2[:, G2R:G2R + 1])
                nc.vector.tensor_copy(out=as2own[:, b:b + 1], in_=ps2[:, 0:1])
                nc.vector.tensor_copy(out=h2own[:, b * NC:(b + 1) * NC], in_=ps2[:, 1:1 + NC])
                t0 += tb

            # ---- halo exchange of layer-2 node table --------------------
            nc.gpsimd.collective_compute(
                "AllGather", ALU.bypass,
                ins=[g2own[:].opt()], outs=[g2full[:].opt()],
                replica_groups=[list(range(CORES))])

            # ---- layer 2 edge phase + epilogue --------------------------
            t0 = 0
            for b in range(NBLK):
                tb = int(T_B[b])
                g2c = gp.tile([P, tb * G2R], F16, tag="g2c")
                for k in range(tb):
                    nc.gpsimd.indirect_dma_start(
                        out=g2c[:, k * G2R:(k + 1) * G2R], out_offset=None,
                        in_=g2full[:, :],
                        in_offset=bass.IndirectOffsetOnAxis(
                            ap=si2[:, t0 + k:t0 + k + 1], axis=0))
                ohc = ohp.tile([P, tb * P], F16, tag="ohc")
                nc.vector.tensor_tensor(
                    out=_ap(ohc, 0, [[P, tb], [1, P]]),
                    in0=_ap(iota_sb, 0, [[0, tb], [1, P]]),
                    in1=_ap(dst_sb, t0, [[1, tb], [0, P]]),
                    op=ALU.is_equal)
                dsr = stp.tile([1, tb * P], F16, tag="dsr")
                nc.sync.dma_start(out=dsr[:], in_=dstrow[0:1, t0 * P:(t0 + tb) * P])
                ohtc = ohtp.tile([P, tb * P], F16, tag="ohtc")
                for g in range(0, tb, 4):
                    gw = min(4, tb - g)
                    bps = psW.tile([P, 4 * P], F32, tag="psW")
                    nc.tensor.matmul(out=bps[:, 0:gw * P], lhsT=ones_sb[:],
                                     rhs=dsr[0:1, g * P:(g + gw) * P],
                                     start=True, stop=True)
                    nc.vector.tensor_tensor(
                        out=ohtc[:, g * P:(g + gw) * P], in0=bps[:, 0:gw * P],
                        in1=_ap(iotac_sb, 0, [[0, gw * P]]),
                        op=ALU.is_equal)
                adps = psB.tile([P, tb], F32, tag="psB")
                for k in range(tb):
                    nc.tensor.matmul(out=adps[:, k:k + 1],
                                     lhsT=ohtc[:, k * P:(k + 1) * P],
                                     rhs=ad2own[:, b:b + 1], start=True, stop=True)
                ech = ep.tile([P, tb], F32, tag="ech")
                nc.vector.tensor_tensor(
                    out=ech[:],
                    in0=_ap(g2c, 0, [[G2R, tb]]),
                    in1=adps[:],
                    op=ALU.add)
                lrch = ep.tile([P, tb], F32, tag="lrch")
                nc.vector.scalar_tensor_tensor(out=lrch[:], in0=ech[:], scalar=0.2,
                                               in1=ech[:], op0=ALU.mult, op1=ALU.max)
                pch = ep.tile([P, tb], F32, tag="pch")
                nc.scalar.activation(pch[:], lrch[:], ACTF.Exp, bias=zcol[:, 0:1])
                vc = vp.tile([P, tb * G2R], F16, tag="vc")
                nc.vector.tensor_copy(out=_ap(vc, 0, [[G2R, tb]]), in_=pch[:])
                for k in range(tb):
                    nc.vector.tensor_scalar_mul(
                        out=vc[:, k * G2R + 1:(k + 1) * G2R],
                        in0=g2c[:, k * G2R + 1:(k + 1) * G2R],
                        scalar1=pch[:, k:k + 1])
                psagg = psA.tile([P, G2R], F32, tag="psA")
                for k in range(tb):
                    nc.tensor.matmul(out=psagg[:], lhsT=ohc[:, k * P:(k + 1) * P],
                                     rhs=vc[:, k * G2R:(k + 1) * G2R],
                                     start=(k == 0), stop=(k == tb - 1))
                es = ep.tile([P, 1], F32, tag="es")
                nc.vector.tensor_tensor(out=es[:], in0=as2own[:, b:b + 1],
                                        in1=ad2own[:, b:b + 1], op=ALU.add)
                lrs = ep.tile([P, 1], F32, tag="lrs")
                nc.vector.scalar_tensor_tensor(out=lrs[:], in0=es[:], scalar=0.2,
                                               in1=es[:], op0=ALU.mult, op1=ALU.max)
                psf = ep.tile([P, 1], F32, tag="psf")
                nc.scalar.activation(psf[:], lrs[:], ACTF.Exp, bias=zcol[:, 0:1])
                st_ = ep.tile([P, 1], F32, tag="st_")
                nc.vector.tensor_tensor(out=st_[:], in0=psagg[:, 0:1], in1=psf[:], op=ALU.add)
                ssb = ep.tile([P, 1], F32, tag="ssb")
                nc.vector.tensor_scalar_add(out=ssb[:], in0=st_[:], scalar1=1e-16)
                sinv = ep.tile([P, 1], F32, tag="sinv")
                nc.vector.reciprocal(out=sinv[:], in_=ssb[:])
                hw2 = ep.tile([P, NC], F32, tag="hw")
                nc.vector.tensor_scalar_mul(out=hw2[:], in0=h2own[:, b * NC:(b + 1) * NC],
                                            scalar1=psf[:, 0:1])
                wf2 = ep.tile([P, NC], F32, tag="wf")
                nc.vector.tensor_tensor(out=wf2[:], in0=psagg[:, 1:1 + NC], in1=hw2[:], op=ALU.add)
                lg = ep.tile([P, NC], F32, tag="t1")
                nc.vector.tensor_scalar_mul(out=lg[:], in0=wf2[:], scalar1=sinv[:, 0:1])
                mx = ep.tile([P, 1], F32, tag="mx")
                nc.vector.reduce_max(mx[:], lg[:], axis=AX.X)
                sh = ep.tile([P, NC], F32, tag="t2")
                nc.vector.tensor_scalar_sub(out=sh[:], in0=lg[:], scalar1=mx[:, 0:1])
                ex = ep.tile([P, NC], F32, tag="t3")
                nc.scalar.activation(ex[:], sh[:], ACTF.Exp, bias=zcol[:, 0:1])
                sm = ep.tile([P, 1], F32, tag="sm")
                nc.vector.reduce_sum(sm[:], ex[:], axis=AX.X)
                ls = ep.tile([P, 1], F32, tag="ls")
                nc.scalar.activation(ls[:], sm[:], ACTF.Ln, bias=zcol[:, 0:1])
                ob = ep.tile([P, NC], F32, tag="t4")
                nc.vector.tensor_scalar_sub(out=ob[:], in0=sh[:], scalar1=ls[:, 0:1])
                nc.sync.dma_start(out=out[b * P:(b + 1) * P, :], in_=ob[:])
                t0 += tb

    nc.compile()
    return nc


def _prep(x, edge_src, edge_dst, W1, a1_src, a1_dst, b1, W2, a2_src, a2_dst, b2):
    """Host-side integer preprocessing (graph partitioning) + param folding."""
    N, F = x.shape
    H, C = a1_src.shape
    NC = W2.shape[1]
    HC = H * C
    NOWN = N // CORES
    NBLK = math.ceil(NOWN / P)
    OWNPAD = NBLK * P
    NFOR = N - NOWN
    FBLK = math.ceil(NFOR / P)
    NODE_BLKS = NBLK + FBLK
    NTAB = NODE_BLKS * P

    # self-loop edges are handled analytically in the block epilogues
    src_all = edge_src
    dst_all = edge_dst

    # per (core, block) edge lists
    core_of = dst_all // NOWN
    per_core = []
    cnt = np.zeros((CORES, NBLK), np.int64)
    for c in range(CORES):
        m = core_of == c
        s, d = src_all[m], dst_all[m] - c * NOWN
        blk = d // P
        order = np.argsort(blk, kind='stable')
        s, d, blk = s[order], d[order], blk[order]
        cnt[c] = np.bincount(blk, minlength=NBLK)
        per_core.append((s, d, blk))
    T_B = np.maximum(1, np.ceil(cnt.max(axis=0) / P).astype(np.int64))
    NT = int(T_B.sum())
    toff = np.concatenate([[0], np.cumsum(T_B)])

    # param folding
    W1r = W1.reshape(F, H, C)
    wsrc = (W1r * a1_src[None]).sum(-1)          # [F, H]
    wdst = (W1r * a1_dst[None]).sum(-1)          # [F, H]
    w1aug = np.concatenate([wsrc, W1, wdst], axis=1).astype(np.float16)   # [F, 8+HC+8]
    b1aug = np.zeros((1, 8 + HC + 8), np.float16)
    b1aug[0, 8:8 + HC] = b1.astype(np.float16)
    G2W = 1 + NC + 1 + 6
    W2K = HC + 1
    w2aug = np.zeros((W2K, G2W), np.float16)
    w2aug[0:HC, 0] = (W2 @ a2_src[0]).astype(np.float16)
    w2aug[0:HC, 1:1 + NC] = W2.astype(np.float16)
    w2aug[0:HC, 1 + NC] = (W2 @ a2_dst[0]).astype(np.float16)
    w2aug[HC, 1:1 + NC] = b2.astype(np.float16)
    ones1 = np.ones((1, P), np.float16)
    iotaf = np.tile(np.arange(P, dtype=np.float32)[None, :], (P, 1))
    ident = np.eye(P, dtype=np.float16)

    xT = np.ascontiguousarray(x.T)               # [F, N] float32

    in_maps = []
    for c in range(CORES):
        own_lo, own_hi = c * NOWN, (c + 1) * NOWN
        # perm: table position -> node
        xTp = np.zeros((F, NTAB), np.float16)
        xTp[:, 0:NOWN] = xT[:, own_lo:own_hi].astype(np.float16)
        fore = np.concatenate([np.arange(0, own_lo), np.arange(own_hi, N)])
        xTp[:, OWNPAD:OWNPAD + NFOR] = xT[:, fore].astype(np.float16)
        # node -> table position
        pos = np.empty(N, np.int64)
        pos[own_lo:own_hi] = np.arange(NOWN)
        pos[fore] = OWNPAD + np.arange(NFOR)

        s, d, blk = per_core[c]
        sidx1 = np.zeros((P, NT), np.int32)
        sidx2 = np.zeros((P, NT), np.int32)
        dstc = np.full((P, NT), -1.0, np.float32)
        bstart = np.concatenate([[0], np.cumsum(np.bincount(blk, minlength=NBLK))])
        for b in range(NBLK):
            eb = slice(bstart[b], bstart[b + 1])
            sb_, db_ = s[eb], d[eb]
            n = len(sb_)
            for k in range(int(T_B[b])):
                lo, hi = k * P, min((k + 1) * P, n)
                if lo >= n:
                    break
                t = toff[b] + k
                m = hi - lo
                sidx1[0:m, t] = pos[sb_[lo:hi]]
                sidx2[0:m, t] = (sb_[lo:hi] // NOWN) * OWNPAD + (sb_[lo:hi] % NOWN)
                dstc[0:m, t] = (db_[lo:hi] % P).astype(np.float32)
        in_maps.append({
            "xT": xTp, "w1aug": w1aug, "b1aug": b1aug, "w2aug": w2aug,
            "ones1": ones1, "iotaf": iotaf.astype(np.float32), "ident": ident,
            "sidx1": sidx1, "sidx2": sidx2, "dstc": dstc,
            "dstrow": dstc.T.reshape(1, NT * P).astype(np.float16),
            "iotac": np.arange(P, dtype=np.float32)[:, None],
        })
    meta = dict(N=N, F=F, H=H, C=C, NC=NC, T_B=T_B, NTAB=NTAB, NBLK=NBLK,
                NODE_BLKS=NODE_BLKS, NOWN=NOWN)
    return in_maps, meta


_CACHED = {}


def run(inputs, eshift=-4.0, trace=False, tmpdir=None):
    in_maps, meta = _prep(**inputs)
    has_b1 = bool(np.any(np.asarray(inputs["b1"])))
    key = (meta["N"], meta["F"], meta["NC"], tuple(meta["T_B"]), has_b1)
    if key not in _CACHED:
        _CACHED[key] = _build_program(meta["N"], meta["F"], meta["H"], meta["C"],
                                      meta["NC"], meta["T_B"], meta["NTAB"],
                                      meta["NBLK"], meta["NODE_BLKS"], eshift, has_b1)
    nc = _CACHED[key]
    kw = {"tmpdir": tmpdir} if tmpdir else {}
    res = bass_utils.run_bass_kernel_spmd(nc, in_maps,
                                          core_ids=list(range(CORES)),
                                          trace=trace, **kw)
    outs = [res.results[c]["out"][:meta["NOWN"]] for c in range(CORES)]
    full = np.concatenate(outs, axis=0).astype(np.float32)
    return full, res


def kernel(**inputs):
    full, _ = run(inputs)
    return full
